# revision 1
# baseline (speedup 1.0000x reference)
"""Trainium2 Bass kernel for LorentzMultiheadAttention (B=2, N=2048, H=8, D=64, E=512).

Sharding: 8 cores = 2 batches x 4 head-pairs. Core c handles batch b=c//4 and
heads {2*(c%4), 2*(c%4)+1}. Each core computes its 2 heads' attention +
per-head centroid, sums them, then a 4-core ReduceScatter sums over all 8
heads of the batch and hands each core a 512-query slice for the final
centroid. Host only marshals layouts (transpose/pad/slice) and concatenates
the output slices.

Math notes:
- The Lorentz centroid  sqrt(C) * x / sqrt(|<x,x>_L|)  is scale-invariant, so
  the softmax denominator and the mean-over-heads divide both cancel: we feed
  unnormalized sum_m exp(att) * v and the plain head-sum into the centroid.
- The Lorentz sign (negated time component) is folded into the kernel by
  negating the K spatial projection weights on the host and negating the
  activation scale of the exp: scores S' = t_q*t_k - q_s.k_s = -L, and
  softmax(att) uses exp(-(2/s)*S' + (2/s + bias)).
- No max-subtraction in softmax: |att| <= ~2 for this problem's scale.
"""

import os
import sys

for _p in ("/opt/trn_rl_repo", "/root/.axon_site/_ro/trn_rl_repo"):
    if os.path.isdir(_p) and _p not in sys.path:
        sys.path.insert(0, _p)

import numpy as np

import concourse.bacc as bacc
import concourse.bass as bass
import concourse.mybir as mybir
import concourse.tile as tile

B = 2
N = 2048
H = 8
D = 64
E = 512
DM1 = D - 1  # 63
P = 128
N_CORES = 8
HPC = 2  # heads per core
QB = N // 4  # 512: query rows output per core

F32 = mybir.dt.float32
BF16 = mybir.dt.bfloat16
EXP = mybir.ActivationFunctionType.Exp
SQRT = mybir.ActivationFunctionType.Sqrt
ADD = mybir.AluOpType.add
MULT = mybir.AluOpType.mult

REPLICA_GROUPS = [[0, 1, 2, 3], [4, 5, 6, 7]]


def _emit(tc, nc, io, scale_val, bias_val):
    """Emit the per-core Tile program. io: dict of DRAM handles."""
    from contextlib import ExitStack

    ctx = ExitStack()
    with ctx:
        consts = ctx.enter_context(tc.tile_pool(name="consts", bufs=1))
        sb = ctx.enter_context(tc.tile_pool(name="sb", bufs=1))
        ctxA = ExitStack()
        psA = ctxA.enter_context(tc.tile_pool(name="psA", bufs=1, space="PSUM"))
        psT = ctxA.enter_context(tc.tile_pool(name="psT", bufs=4, space="PSUM"))

        # ---- constants / weights to SBUF ----
        ident = consts.tile([P, P], BF16)
        nc.sync.dma_start(ident[:], io["ident"].ap())
        ident2 = consts.tile([P, 64], F32)
        nc.sync.dma_start(ident2[:], io["ident2"].ap())
        mask65 = consts.tile([P, 65], F32)
        nc.sync.dma_start(mask65[:], io["mask65"].ap())

        w_sb = {}
        b_sb = {}
        for nm in ("wq", "wk", "wv"):
            w = consts.tile([P, 4, P], BF16, name=f"{nm}_sb")
            nc.sync.dma_start(w[:], io[nm].ap().rearrange("(c p) m -> p c m", p=P))
            w_sb[nm] = w
        for nm in ("bq", "bk", "bv"):
            bt = consts.tile([P, 1], F32, name=f"{nm}_sb")
            nc.sync.dma_start(bt[:], io[nm].ap().rearrange("(p one) -> p one", one=1))
            b_sb[nm] = bt

        xq = sb.tile([P, 4, N], BF16)
        nc.sync.dma_start(xq[:], io["xq_t"].ap().rearrange("(c p) n -> p c n", p=P))
        xs = sb.tile([P, 4, N], BF16)
        nc.sync.dma_start(xs[:], io["xs_t"].ap().rearrange("(c p) n -> p c n", p=P))

        # ---- Phase A: projections (transposed layout, d on partitions) ----
        # qsT/ksT: [128, N]; rows 0..63 = head0 [t, 63 spatial], 64..127 head1.
        qsT = sb.tile([P, N], BF16)
        ksT = sb.tile([P, N], BF16)
        vT = sb.tile([P, N], BF16)

        def project(dst, x_sb, w, bias):
            ps = psA.tile([P, N], F32, tag="projps")
            for qc in range(4):
                for ec in range(4):
                    nc.tensor.matmul(
                        ps[:, qc * 512 : (qc + 1) * 512],
                        lhsT=w[:, ec, :],
                        rhs=x_sb[:, ec, qc * 512 : (qc + 1) * 512],
                        start=(ec == 0),
                        stop=(ec == 3),
                    )
            # psum -> sbuf with per-partition bias add
            nc.vector.tensor_tensor(
                dst[:], ps[:], bias[:].to_broadcast((P, N)), ADD
            )

        project(qsT, xq, w_sb["wq"], b_sb["bq"])
        project(ksT, xs, w_sb["wk"], b_sb["bk"])
        project(vT, xs, w_sb["wv"], b_sb["bv"])

        # ---- lift q, k: time row t = sqrt(1 + sum spatial^2) at rows 0/64 ----
        def lift_T(dst):
            sq = sb.tile([P, N], F32, tag="liftsq")
            nc.vector.tensor_tensor(sq[:], dst[:], dst[:], MULT)
            nrm = psA.tile([65, N], F32, tag="projps")
            for qc in range(4):
                nc.tensor.matmul(
                    nrm[:, qc * 512 : (qc + 1) * 512],
                    lhsT=mask65[:],
                    rhs=sq[:, qc * 512 : (qc + 1) * 512],
                    start=True,
                    stop=True,
                )
            nc.scalar.activation(dst[0:1, :], nrm[0:1, :], SQRT, bias=1.0, scale=1.0)
            nc.scalar.activation(dst[64:65, :], nrm[64:65, :], SQRT, bias=1.0, scale=1.0)

        lift_T(qsT)
        lift_T(ksT)

        # ---- V to natural layout [m, d] via PE transpose; lift per row ----
        # v_sb: [128 m-part, 16 m-tiles, 128 (h*64 + d)]
        v_sb = sb.tile([P, 16, P], BF16)
        for mt in range(16):
            pt = psT.tile([P, P], BF16, tag="vtp")
            nc.tensor.transpose(pt[:], vT[:, mt * P : (mt + 1) * P], ident[:])
            nc.vector.tensor_copy(out=v_sb[:, mt, :], in_=pt[:])
        vsq = sb.tile([P, 16, P], F32, tag="liftsq2")
        nc.vector.tensor_tensor(vsq[:], v_sb[:], v_sb[:], MULT)
        vn = sb.tile([P, 16, 2, 1], F32)
        nc.vector.tensor_reduce(
            vn[:, :, :, 0],
            vsq[:].rearrange("p t (h d) -> p t h d", h=2),
            axis=mybir.AxisListType.X,
            op=ADD,
        )
        # write time cols 0 and 64: v_t = sqrt(1 + ||v_s||^2)
        nc.scalar.activation(
            v_sb[:].rearrange("p t (h d) -> p t h d", h=2)[:, :, :, 0:1],
            vn[:],
            SQRT,
            bias=1.0,
            scale=1.0,
        )

        # ---- Phase B: attention (scores transposed: [keys, queries]) ----
        ctxA.close()  # free phase-A PSUM banks
        ctxB = ExitStack()
        psS = ctxB.enter_context(tc.tile_pool(name="psS", bufs=2, space="PSUM"))
        psPV = ctxB.enter_context(tc.tile_pool(name="psPV", bufs=1, space="PSUM"))
        pP = ctx.enter_context(tc.tile_pool(name="pP", bufs=4))

        pv_tiles = [psPV.tile([P, 512], F32, name=f"pv{qc}") for qc in range(4)]

        act_scale = -2.0 / scale_val
        act_bias = 2.0 / scale_val + bias_val
        ebias = consts.tile([P, 1], F32)
        nc.vector.memset(ebias[:], act_bias)

        for mc in range(16):
            for h in range(HPC):
                kT_sl = ksT[h * 64 : (h + 1) * 64, mc * P : (mc + 1) * P]
                for qh in range(2):
                    s_ps = psS.tile([P, 1024], F32, tag="s")
                    for qq in range(2):
                        q0 = qh * 1024 + qq * 512
                        nc.tensor.matmul(
                            s_ps[:, qq * 512 : (qq + 1) * 512],
                            lhsT=kT_sl,
                            rhs=qsT[h * 64 : (h + 1) * 64, q0 : q0 + 512],
                            start=True,
                            stop=True,
                        )
                    p_sb = pP.tile([P, 1024], BF16, tag="p")
                    nc.scalar.activation(
                        p_sb[:], s_ps[:], EXP, scale=act_scale, bias=ebias[:]
                    )
                    for qq in range(2):
                        qc = qh * 2 + qq
                        nc.tensor.matmul(
                            pv_tiles[qc][h * 64 : (h + 1) * 64, :],
                            lhsT=v_sb[:, mc, h * 64 : (h + 1) * 64],
                            rhs=p_sb[:, qq * 512 : (qq + 1) * 512],
                            start=(mc == 0),
                            stop=(mc == 15),
                            # two col-packed head groups share each PSUM bank;
                            # their element ranges are disjoint
                            skip_group_check=True,
                        )

        # ---- per-head centroid + head-sum (natural layout) ----
        o_unT = sb.tile([P, N], F32)
        for qc in range(4):
            nc.vector.tensor_copy(
                out=o_unT[:, qc * 512 : (qc + 1) * 512], in_=pv_tiles[qc][:]
            )
        ctxB.close()  # free phase-B PSUM banks
        psO = ctx.enter_context(tc.tile_pool(name="psO", bufs=4, space="PSUM"))
        o_nat = sb.tile([P, 16, P], F32)  # [q-part, q-tile, h*64+d]
        for h in range(HPC):
            for qt in range(16):
                pt = psO.tile([P, 64], F32, tag="otp")
                nc.tensor.transpose(
                    pt[:],
                    o_unT[h * 64 : (h + 1) * 64, qt * P : (qt + 1) * P],
                    ident2[h * 64 : (h + 1) * 64, :],
                )
                nc.vector.tensor_copy(out=o_nat[:, qt, h * 64 : (h + 1) * 64], in_=pt[:])

        def centroid_scale(src, n_t, tag):
            """src: [P, n_t, 2, 64] view-able sbuf tile -> per-(row,tile,h)
            1/sqrt(|inner|) in an [P, n_t, 2, 1] tile."""
            v4 = src[:].rearrange("p t (h d) -> p t h d", h=2)
            sq = sb.tile([P, n_t, P], F32, tag=f"{tag}_sq")
            nc.vector.tensor_tensor(sq[:], src[:], src[:], MULT)
            ssum = sb.tile([P, n_t, 2, 1], F32, tag=f"{tag}_ss")
            nc.vector.tensor_reduce(
                ssum[:, :, :, 0],
                sq[:].rearrange("p t (h d) -> p t h d", h=2),
                axis=mybir.AxisListType.X,
                op=ADD,
            )
            t2 = sb.tile([P, n_t, 2, 1], F32, tag=f"{tag}_t2")
            nc.vector.tensor_tensor(t2[:], v4[:, :, :, 0:1], v4[:, :, :, 0:1], MULT)
            nc.vector.tensor_scalar_mul(t2[:], t2[:], -2.0)
            nc.vector.tensor_tensor(ssum[:], ssum[:], t2[:], ADD)  # = inner (<0)
            den = sb.tile([P, n_t, 2, 1], F32, tag=f"{tag}_den")
            nc.scalar.activation(den[:], ssum[:], SQRT, bias=0.0, scale=-1.0)
            rec = sb.tile([P, n_t, 2, 1], F32, tag=f"{tag}_rec")
            nc.vector.reciprocal(rec[:], den[:])
            return rec

        rec = centroid_scale(o_nat, 16, "ph")
        o4 = o_nat[:].rearrange("p t (h d) -> p t h d", h=2)
        part0 = sb.tile([P, 16, D], F32)
        part1 = sb.tile([P, 16, D], F32)
        nc.vector.tensor_tensor(
            part0[:], o4[:, :, 0, :], rec[:, :, 0, :].to_broadcast((P, 16, D)), MULT
        )
        nc.vector.tensor_tensor(
            part1[:], o4[:, :, 1, :], rec[:, :, 1, :].to_broadcast((P, 16, D)), MULT
        )
        nc.vector.tensor_tensor(part0[:], part0[:], part1[:], ADD)

        # ---- ReduceScatter over the 4-core batch group ----
        dram = ctx.enter_context(tc.tile_pool(name="dram", bufs=1, space="DRAM"))
        cc_in = dram.tile([N, D], F32)
        cc_out = dram.tile([QB, D], F32)
        nc.sync.dma_start(
            cc_in[:].rearrange("(t p) d -> p t d", p=P), part0[:]
        )
        nc.gpsimd.collective_compute(
            "ReduceScatter",
            ADD,
            replica_groups=REPLICA_GROUPS,
            ins=[cc_in[:].opt()],
            outs=[cc_out[:].opt()],
        )

        # ---- final centroid on the local 512-query slice ----
        # [P, 2, 128] tile; natural [P, 4, 64] view groups of 2 per free-row
        fin = sb.tile([P, 2, P], F32)
        nc.sync.dma_start(
            fin[:].rearrange("p t (g d) -> p (t g) d", g=2),
            cc_out[:].rearrange("(t p) d -> p t d", p=P),
        )
        rec2 = centroid_scale(fin, 2, "fin")
        f4 = fin[:].rearrange("p t (h d) -> p t h d", h=2)
        out_sb = sb.tile([P, 2, P], F32)
        ov = out_sb[:].rearrange("p t (h d) -> p t h d", h=2)
        nc.vector.tensor_tensor(
            ov[:, :, 0, :], f4[:, :, 0, :], rec2[:, :, 0, :].to_broadcast((P, 2, D)), MULT
        )
        nc.vector.tensor_tensor(
            ov[:, :, 1, :], f4[:, :, 1, :], rec2[:, :, 1, :].to_broadcast((P, 2, D)), MULT
        )
        nc.sync.dma_start(
            io["out"].ap().rearrange("(t p) d -> p t d", p=P),
            out_sb[:].rearrange("p t (g d) -> p (t g) d", g=2),
        )


def _build(scale_val, bias_val):
    nc = bacc.Bacc(num_devices=N_CORES)
    io = {}
    io["xq_t"] = nc.declare_dram_parameter("xq_t", [E, N], BF16, isOutput=False)
    io["xs_t"] = nc.declare_dram_parameter("xs_t", [E, N], BF16, isOutput=False)
    for nm in ("wq", "wk"):
        io[nm] = nc.declare_dram_parameter(nm, [E, P], BF16, isOutput=False)
    io["wv"] = nc.declare_dram_parameter("wv", [E, P], BF16, isOutput=False)
    for nm in ("bq", "bk", "bv"):
        io[nm] = nc.declare_dram_parameter(nm, [P], F32, isOutput=False)
    io["ident"] = nc.declare_dram_parameter("ident", [P, P], BF16, isOutput=False)
    io["ident2"] = nc.declare_dram_parameter("ident2", [P, 64], F32, isOutput=False)
    io["mask65"] = nc.declare_dram_parameter("mask65", [P, 65], F32, isOutput=False)
    io["out"] = nc.declare_dram_parameter("out", [QB, D], F32, isOutput=True)

    with tile.TileContext(nc) as tc:
        _emit(tc, nc, io, scale_val, bias_val)
    nc.compile()
    return nc


_BUILD_CACHE = {}


def _get_nc(scale_val, bias_val):
    key = (float(scale_val), float(bias_val))
    if key not in _BUILD_CACHE:
        _BUILD_CACHE[key] = _build(*key)
    return _BUILD_CACHE[key]


def _pad_wT(w_heads):
    """w_heads: [126, 512] spatial weights for 2 heads -> [512, 128] transposed
    with zero columns at 0 and 64 (time slots)."""
    out = np.zeros((E, P), dtype=np.float32)
    out[:, 1:64] = w_heads[0:DM1, :].T
    out[:, 65:128] = w_heads[DM1 : 2 * DM1, :].T
    return np.ascontiguousarray(out)


def _pad_b(b_heads):
    out = np.zeros((P,), dtype=np.float32)
    out[1:64] = b_heads[0:DM1]
    out[65:128] = b_heads[DM1 : 2 * DM1]
    return out


def make_in_maps(
    query_input, source_input, Wq_w, Wq_b, Wk_w, Wk_b, Wv_w, Wv_b, scale, bias
):
    import ml_dtypes

    BF = ml_dtypes.bfloat16
    ident = np.eye(P, dtype=BF)
    ident2 = np.concatenate([np.eye(64), np.eye(64)], axis=0).astype(np.float32)
    mask65 = np.zeros((P, 65), dtype=np.float32)
    mask65[1:64, 0] = 1.0
    mask65[65:128, 64] = 1.0

    in_maps = []
    for c in range(N_CORES):
        b = c // 4
        h0 = 2 * (c % 4)
        sl = slice(h0 * DM1, (h0 + 2) * DM1)
        m = {
            "xq_t": np.ascontiguousarray(query_input[b].T).astype(BF),
            "xs_t": np.ascontiguousarray(source_input[b].T).astype(BF),
            "wq": _pad_wT(Wq_w[sl]).astype(BF),
            "wk": _pad_wT(-Wk_w[sl]).astype(BF),  # Lorentz sign folded into K
            "wv": _pad_wT(Wv_w[sl]).astype(BF),
            "bq": _pad_b(Wq_b[sl]),
            "bk": _pad_b(-Wk_b[sl]),
            "bv": _pad_b(Wv_b[sl]),
            "ident": ident,
            "ident2": ident2,
            "mask65": mask65,
        }
        in_maps.append(m)
    return in_maps


def kernel(
    query_input,
    source_input,
    Wq_w,
    Wq_b,
    Wk_w,
    Wk_b,
    Wv_w,
    Wv_b,
    scale,
    bias,
    _trace=False,
):
    scale_val = float(np.asarray(scale).reshape(-1)[0])
    bias_val = float(np.asarray(bias).reshape(-1)[0]) if np.asarray(bias).size else 0.0

    nc = _get_nc(scale_val, bias_val)
    in_maps = make_in_maps(
        query_input, source_input, Wq_w, Wq_b, Wk_w, Wk_b, Wv_w, Wv_b, scale, bias
    )

    from concourse.bass_utils import run_bass_kernel_spmd

    res = run_bass_kernel_spmd(
        nc, in_maps, core_ids=list(range(N_CORES)), trace=_trace
    )

    out = np.zeros((B, N, D), dtype=np.float32)
    for c in range(N_CORES):
        b = c // 4
        g = c % 4
        out[b, g * QB : (g + 1) * QB, :] = res.results[c]["out"]
    if _trace:
        kernel.last_exec_time_ns = res.exec_time_ns
        kernel.last_results = res
    return out



# revision 13
# speedup vs baseline: 1.1164x; 1.1164x over previous
"""Trainium2 Bass kernel for LorentzMultiheadAttention (B=2, N=2048, H=8, D=64, E=512).

Sharding: 8 cores = 2 batches x 4 query-quarters. Core c handles batch b=c//4
and queries [512*(c%4), 512*(c%4+1)) for ALL 8 heads. K/V projections are
recomputed on each core of a batch group (cheaper than the inter-core
ReduceScatter they replace) so the kernel has NO collectives at all: the final
per-head centroid, head mean, and second centroid are all core-local.

Math notes:
- The Lorentz centroid  sqrt(C) * x / sqrt(|<x,x>_L|)  is scale-invariant, so
  the softmax denominator and the mean-over-heads divide both cancel: we feed
  unnormalized sum_m exp(att) * v and the plain head-sum into the centroid.
- The Lorentz sign (negated time component) is folded into the kernel by
  negating the K projection weights on the host and negating the activation
  scale of the exp: scores S' = t_q*t_k - q_s.k_s = -<q,k>_L, and
  softmax(att) uses exp(-(2/s)*S' + (2/s + bias)).
- No max-subtraction in softmax: |att| <= ~3 for this problem's scale.
"""

import os
import sys

for _p in ("/opt/trn_rl_repo", "/root/.axon_site/_ro/trn_rl_repo"):
    if os.path.isdir(_p) and _p not in sys.path:
        sys.path.insert(0, _p)

import numpy as np

import concourse.bacc as bacc
import concourse.bass as bass
import concourse.mybir as mybir
import concourse.tile as tile

B = 2
N = 2048
H = 8
D = 64
E = 512
DM1 = D - 1  # 63
P = 128
N_CORES = 8
QB = N // 4  # 512 queries per core
NHP = 4  # head-pairs per core

F32 = mybir.dt.float32
BF16 = mybir.dt.bfloat16
EXP = mybir.ActivationFunctionType.Exp
SQRT = mybir.ActivationFunctionType.Sqrt
ADD = mybir.AluOpType.add
MULT = mybir.AluOpType.mult


def _emit(tc, nc, io, scale_val, bias_val):
    from contextlib import ExitStack

    ctx = ExitStack()
    with ctx:
        consts = ctx.enter_context(tc.tile_pool(name="consts", bufs=1))
        sb = ctx.enter_context(tc.tile_pool(name="sb", bufs=1))
        scr = ctx.enter_context(tc.tile_pool(name="scr", bufs=2))
        pP = ctx.enter_context(tc.tile_pool(name="pP", bufs=4))

        ctxPro = ExitStack()
        psU = ctxPro.enter_context(tc.tile_pool(name="psU", bufs=2, space="PSUM"))

        # ---- constants / weights ----
        ident2 = consts.tile([P, 64], F32)
        nc.sync.dma_start(ident2[:], io["ident2"].ap())
        mask2 = consts.tile([P, 2], F32)
        nc.sync.dma_start(mask2[:], io["mask2"].ap())

        w_sb = {}
        b_sb = {}
        for nm in ("wq", "wk", "wv"):
            w = consts.tile([P, 4, 4, P], BF16, name=f"{nm}_sb")
            nc.sync.dma_start(
                w[:], io[nm].ap().rearrange("(c p) (t m) -> p c t m", p=P, m=P)
            )
            w_sb[nm] = w
        for nm in ("bq", "bk", "bv"):
            bt = consts.tile([P, 4], F32, name=f"{nm}_sb")
            nc.sync.dma_start(bt[:], io[nm].ap().rearrange("(t p) -> p t", p=P))
            b_sb[nm] = bt

        ebias = consts.tile([P, 1], F32)
        nc.vector.memset(ebias[:], 2.0 / scale_val + bias_val)

        xq = sb.tile([P, 4, QB], BF16)
        nc.sync.dma_start(xq[:], io["xq_t"].ap().rearrange("(c p) n -> p c n", p=P))
        xs = sb.tile([P, 4, N], BF16)
        nc.sync.dma_start(xs[:], io["xs_t"].ap().rearrange("(c p) n -> p c n", p=P))

        # ---- projections (transposed layout: d on partitions) ----
        # qsT: [128, hp, 512]; rows 0..63 = even head [t, 63 spatial], 64..127 odd.
        qsT = sb.tile([P, NHP, QB], BF16)
        ksT = sb.tile([P, NHP, N], BF16)
        vT = sb.tile([P, NHP, N], BF16)

        def project(dst_sl, x_sl, w, pt, bias, ncols):
            """dst_sl/x_sl: [128, ncols] sbuf views; accumulate 4 E-chunks."""
            for qc in range(ncols // 512):
                ps = psU.tile([P, 512], F32, tag="proj")
                for ec in range(4):
                    nc.tensor.matmul(
                        ps[:],
                        lhsT=w[:, ec, pt, :],
                        rhs=x_sl[:, ec, qc * 512 : (qc + 1) * 512],
                        start=(ec == 0),
                        stop=(ec == 3),
                    )
                nc.vector.tensor_tensor(
                    dst_sl[:, qc * 512 : (qc + 1) * 512],
                    ps[:],
                    bias.to_broadcast((P, 512)),
                    ADD,
                )

        # Q projection + lift
        for hp in range(NHP):
            project(qsT[:, hp, :], xq, w_sb["wq"], hp, b_sb["bq"][:, hp : hp + 1], QB)
        qsq = sb.tile([P, NHP, QB], F32)
        nc.vector.tensor_tensor(qsq[:], qsT[:], qsT[:], MULT)
        for hp in range(NHP):
            nrm = psU.tile([2, 512], F32, tag="nrm")
            nc.tensor.matmul(
                nrm[:], lhsT=mask2[:], rhs=qsq[:, hp, :], start=True, stop=True
            )
            # head sums land contiguously on partitions {0,1}: one sqrt, then a
            # small DMA scatters the two time rows to partitions {0,64}.
            qt_s = scr.tile([2, 512], BF16, tag="qts")
            nc.scalar.activation(qt_s[:], nrm[:], SQRT, bias=1.0, scale=1.0)
            nc.sync.dma_start(qsT[0:65:64, hp, :], qt_s[:])

        # K/V projections + K lift (in transposed layout)
        for hp in range(NHP):
            project(ksT[:, hp, :], xs, w_sb["wk"], hp, b_sb["bk"][:, hp : hp + 1], N)
            project(vT[:, hp, :], xs, w_sb["wv"], hp, b_sb["bv"][:, hp : hp + 1], N)
        for hp in range(NHP):
            ksq = scr.tile([P, N], F32, tag="ksq")
            nc.vector.tensor_tensor(ksq[:], ksT[:, hp, :], ksT[:, hp, :], MULT)
            kt_s = scr.tile([2, 4, 512], BF16, tag="kts")
            for qc in range(4):
                nrm = psU.tile([2, 512], F32, tag="nrm")
                nc.tensor.matmul(
                    nrm[:],
                    lhsT=mask2[:],
                    rhs=ksq[:, qc * 512 : (qc + 1) * 512],
                    start=True,
                    stop=True,
                )
                nc.scalar.activation(
                    kt_s[:, qc, :], nrm[:], SQRT, bias=1.0, scale=1.0
                )
            nc.sync.dma_start(
                ksT[0:65:64, hp, :], kt_s[:].rearrange("t c n -> t (c n)")
            )

        # V to natural layout [keys, h*64+d] via DMA xbar transpose, then lift.
        v_nat = sb.tile([P, 16, 4, P], BF16)  # [key%128, keytile, hp, 2h*64]
        for hp in range(NHP):
            for kt in range(16):
                nc.sync.dma_start(
                    v_nat[:, kt, hp, :],
                    vT[:, hp, kt * P : (kt + 1) * P],
                    transpose=True,
                )
        vsq = sb.tile([P, 16, 4, P], F32)
        nc.vector.tensor_tensor(vsq[:], v_nat[:], v_nat[:], MULT)
        vn = sb.tile([P, 16, 8, 1], F32)
        nc.vector.tensor_reduce(
            vn[:, :, :, 0],
            vsq[:].rearrange("p t hp (h d) -> p t (hp h) d", h=2),
            axis=mybir.AxisListType.X,
            op=ADD,
        )
        nc.scalar.activation(
            v_nat[:].rearrange("p t hp (h d) -> p t (hp h) d", h=2)[:, :, :, 0:1],
            vn[:],
            SQRT,
            bias=1.0,
            scale=1.0,
        )

        # ---- attention: all 8 heads, PSUM = 4 PV banks + 2x2 score banks ----
        ctxPro.close()
        ctxA = ExitStack()
        psS = ctxA.enter_context(tc.tile_pool(name="psS", bufs=2, space="PSUM"))
        psPV = ctxA.enter_context(tc.tile_pool(name="psPV", bufs=1, space="PSUM"))
        act_scale = -2.0 / scale_val
        pv_tiles = [psPV.tile([P, QB], F32, name=f"pv{hp}") for hp in range(NHP)]

        for mc in range(16):
            for hp in range(NHP):
                s_ps = psS.tile([P, 1024], F32, tag="s")
                for h in range(2):
                    nc.tensor.matmul(
                        s_ps[:, h * 512 : (h + 1) * 512],
                        lhsT=ksT[h * 64 : (h + 1) * 64, hp, mc * P : (mc + 1) * P],
                        rhs=qsT[h * 64 : (h + 1) * 64, hp, :],
                        start=True,
                        stop=True,
                    )
                p_sb = pP.tile([P, 1024], BF16, tag="p")
                nc.scalar.activation(
                    p_sb[:], s_ps[:], EXP, scale=act_scale, bias=ebias[:]
                )
                for h in range(2):
                    nc.tensor.matmul(
                        pv_tiles[hp][h * 64 : (h + 1) * 64, :],
                        lhsT=v_nat[:, mc, hp, h * 64 : (h + 1) * 64],
                        rhs=p_sb[:, h * 512 : (h + 1) * 512],
                        start=(mc == 0),
                        stop=(mc == 15),
                        skip_group_check=True,
                    )

        # ---- drain PV, free attention PSUM, transpose to natural layout ----
        o_unT = sb.tile([P, NHP, QB], F32)
        for hp in range(NHP):
            nc.vector.tensor_copy(out=o_unT[:, hp, :], in_=pv_tiles[hp][:])
        ctxA.close()
        psO = ctx.enter_context(tc.tile_pool(name="psO", bufs=4, space="PSUM"))

        o_nat = sb.tile([P, 4, H, D], F32)  # [q%128, qtile, head, d]
        for h in range(H):
            hp, hh = h // 2, h % 2
            for qt in range(4):
                pt = psO.tile([P, 64], F32, tag="otp")
                nc.tensor.transpose(
                    pt[:],
                    o_unT[hh * 64 : (hh + 1) * 64, hp, qt * P : (qt + 1) * P],
                    ident2[hh * 64 : (hh + 1) * 64, :],
                )
                nc.vector.tensor_copy(out=o_nat[:, qt, h, :], in_=pt[:])

        # ---- per-head centroid scale, head-sum, final centroid ----
        def centroid_scale(src, nt, nh, tag):
            """src: [P, nt, nh, 64] f32 -> rec [P, nt, nh, 1] = 1/sqrt(|inner|)."""
            sq = sb.tile([P, nt, nh, D], F32, name=f"{tag}_sq")
            nc.vector.tensor_tensor(sq[:], src[:], src[:], MULT)
            ssum = sb.tile([P, nt, nh, 1], F32, name=f"{tag}_ss")
            nc.vector.tensor_reduce(
                ssum[:, :, :, 0], sq[:], axis=mybir.AxisListType.X, op=ADD
            )
            t2 = sb.tile([P, nt, nh, 1], F32, name=f"{tag}_t2")
            nc.vector.tensor_tensor(t2[:], src[:, :, :, 0:1], src[:, :, :, 0:1], MULT)
            nc.vector.tensor_scalar_mul(t2[:], t2[:], -2.0)
            nc.vector.tensor_tensor(ssum[:], ssum[:], t2[:], ADD)  # = inner (<0)
            den = sb.tile([P, nt, nh, 1], F32, name=f"{tag}_den")
            nc.scalar.activation(den[:], ssum[:], SQRT, bias=0.0, scale=-1.0)
            rec = sb.tile([P, nt, nh, 1], F32, name=f"{tag}_rec")
            nc.vector.reciprocal(rec[:], den[:])
            return rec

        rec = centroid_scale(o_nat, 4, H, "ph")
        cent = sb.tile([P, 4, H, D], F32)
        nc.vector.tensor_tensor(
            cent[:], o_nat[:], rec[:].to_broadcast((P, 4, H, D)), MULT
        )
        hsum = sb.tile([P, 4, 1, D], F32)
        nc.vector.tensor_reduce(
            hsum[:].rearrange("p t one d -> p t (one d)"),
            cent[:].rearrange("p t h d -> p t d h"),
            axis=mybir.AxisListType.X,
            op=ADD,
        )
        rec2 = centroid_scale(hsum, 4, 1, "fin")
        out_sb = sb.tile([P, 4, D], F32)
        nc.vector.tensor_tensor(
            out_sb[:],
            hsum[:, :, 0, :],
            rec2[:, :, 0, :].to_broadcast((P, 4, D)),
            MULT,
        )
        nc.sync.dma_start(
            io["out"].ap().rearrange("(t p) d -> p t d", p=P), out_sb[:]
        )


def _build(scale_val, bias_val):
    nc = bacc.Bacc(num_devices=N_CORES)
    io = {}
    io["xq_t"] = nc.declare_dram_parameter("xq_t", [E, QB], BF16, isOutput=False)
    io["xs_t"] = nc.declare_dram_parameter("xs_t", [E, N], BF16, isOutput=False)
    for nm in ("wq", "wk", "wv"):
        io[nm] = nc.declare_dram_parameter(nm, [E, 512], BF16, isOutput=False)
    for nm in ("bq", "bk", "bv"):
        io[nm] = nc.declare_dram_parameter(nm, [512], F32, isOutput=False)
    io["ident2"] = nc.declare_dram_parameter("ident2", [P, 64], F32, isOutput=False)
    io["mask2"] = nc.declare_dram_parameter("mask2", [P, 2], F32, isOutput=False)
    io["out"] = nc.declare_dram_parameter("out", [QB, D], F32, isOutput=True)

    with tile.TileContext(nc) as tc:
        _emit(tc, nc, io, scale_val, bias_val)
    nc.compile()
    return nc


_BUILD_CACHE = {}


def _get_nc(scale_val, bias_val):
    key = (float(scale_val), float(bias_val))
    if key not in _BUILD_CACHE:
        _BUILD_CACHE[key] = _build(*key)
    return _BUILD_CACHE[key]


def _pad_wT8(w):
    """w: [504, 512] spatial weights for 8 heads -> [512, 512] transposed with
    zero columns at each head's time slot (col h*64)."""
    out = np.zeros((E, 512), dtype=np.float32)
    for h in range(H):
        out[:, h * 64 + 1 : (h + 1) * 64] = w[h * DM1 : (h + 1) * DM1, :].T
    return np.ascontiguousarray(out)


def _pad_b8(b):
    out = np.zeros((512,), dtype=np.float32)
    for h in range(H):
        out[h * 64 + 1 : (h + 1) * 64] = b[h * DM1 : (h + 1) * DM1]
    return out


def make_in_maps(
    query_input, source_input, Wq_w, Wq_b, Wk_w, Wk_b, Wv_w, Wv_b, scale, bias
):
    import ml_dtypes

    BF = ml_dtypes.bfloat16
    ident2 = np.concatenate([np.eye(64), np.eye(64)], axis=0).astype(np.float32)
    mask2 = np.zeros((P, 2), dtype=np.float32)
    mask2[1:64, 0] = 1.0
    mask2[65:128, 1] = 1.0

    wq = _pad_wT8(Wq_w).astype(BF)
    wk = _pad_wT8(-Wk_w).astype(BF)  # Lorentz sign folded into K
    wv = _pad_wT8(Wv_w).astype(BF)
    bq = _pad_b8(Wq_b)
    bk = _pad_b8(-Wk_b)
    bv = _pad_b8(Wv_b)

    xs_t = [np.ascontiguousarray(source_input[b].T).astype(BF) for b in range(B)]

    in_maps = []
    for c in range(N_CORES):
        b = c // 4
        g = c % 4
        m = {
            "xq_t": np.ascontiguousarray(
                query_input[b, g * QB : (g + 1) * QB, :].T
            ).astype(BF),
            "xs_t": xs_t[b],
            "wq": wq,
            "wk": wk,
            "wv": wv,
            "bq": bq,
            "bk": bk,
            "bv": bv,
            "ident2": ident2,
            "mask2": mask2,
        }
        in_maps.append(m)
    return in_maps


def kernel(
    query_input,
    source_input,
    Wq_w,
    Wq_b,
    Wk_w,
    Wk_b,
    Wv_w,
    Wv_b,
    scale,
    bias,
    _trace=False,
):
    scale_val = float(np.asarray(scale).reshape(-1)[0])
    bias_val = float(np.asarray(bias).reshape(-1)[0]) if np.asarray(bias).size else 0.0

    nc = _get_nc(scale_val, bias_val)
    in_maps = make_in_maps(
        query_input, source_input, Wq_w, Wq_b, Wk_w, Wk_b, Wv_w, Wv_b, scale, bias
    )

    from concourse.bass_utils import run_bass_kernel_spmd

    res = run_bass_kernel_spmd(
        nc, in_maps, core_ids=list(range(N_CORES)), trace=_trace
    )

    out = np.zeros((B, N, D), dtype=np.float32)
    for c in range(N_CORES):
        b = c // 4
        g = c % 4
        out[b, g * QB : (g + 1) * QB, :] = res.results[c]["out"]
    if _trace:
        kernel.last_exec_time_ns = res.exec_time_ns
        kernel.last_results = res
    return out


# revision 24
# speedup vs baseline: 1.5630x; 1.4000x over previous
"""Trainium2 Bass kernel for LorentzMultiheadAttention (B=2, N=2048, H=8, D=64, E=512).

Sharding: 8 cores = 2 batches x 4 query-quarters. Core c handles batch b=c//4
and queries [512*(c%4), 512*(c%4+1)) for ALL 8 heads. K/V projections are
recomputed on each core of a batch group (cheaper than an inter-core
ReduceScatter) so the kernel has NO collectives: per-head centroids, the head
mean, and the second centroid are all core-local.

Structure: a 4-stage pipeline over head-pairs. For each head-pair hp:
project K/V -> lift -> one xbar-transpose of V -> 16 attention iterations
(2 score MMs -> EXP[128,1024] -> 2 PV MMs). The next head-pair's projections
are scheduled into the PE slack under the current pair's (ACT-bound) EXP
stream. PSUM: 4 score banks + 2 PV banks (alternating per hp) + 2 utility
banks (projections / lift sums / output transposes) = 8.

ACT table-set discipline: sqrt(x) is computed as exp(0.5*ln(x)) and
1/sqrt(|x|) as exp(-0.5*ln(-x)) so every ACT instruction (lifts, attention
EXPs, centroid scales) draws from the natural_log_exp table set -- no
mid-stream ACT_TABLE_LOADs.

Layout tricks:
- The [128,2048]->[128,16,128] DMA xbar transpose delivers transposed row r
  (= key index) to partition r%128, free tile r//128 -- exactly the
  natural-layout V that PV matmuls need, in ONE instruction per head-pair.
- Lift time rows: accumulating mask-variant matmuls pack all of a head-pair's
  sum(x_spatial^2) rows (K and V, 4 column chunks, even/odd head) into one
  PSUM bank at rows 0..15, so a single 16-lane Ln+Exp computes every
  t = sqrt(1+s); small SBUF->SBUF DMAs scatter the two time rows of each
  target to partitions {0,64}.

Math notes:
- The Lorentz centroid sqrt(C)*x/sqrt(|<x,x>_L|) is scale-invariant, so the
  softmax denominator and the mean-over-heads divide both cancel; PV feeds
  unnormalized sum_m exp(att)*v into the centroid.
- The Lorentz sign is folded by negating K weights on the host:
  scores S' = t_q*t_k - q_s.k_s = -<q,k>_L and softmax weights are
  exp(-(2/s)*S' + (2/s + bias)). No max-subtraction: |att| <= ~3.
"""

import os
import sys

for _p in ("/opt/trn_rl_repo", "/root/.axon_site/_ro/trn_rl_repo"):
    if os.path.isdir(_p) and _p not in sys.path:
        sys.path.insert(0, _p)

import numpy as np

import concourse.bacc as bacc
import concourse.bass as bass
import concourse.mybir as mybir
import concourse.tile as tile

B = 2
N = 2048
H = 8
D = 64
E = 512
DM1 = D - 1  # 63
P = 128
N_CORES = 8
QB = N // 4  # 512 queries per core
NHP = 4  # head-pairs per core

F32 = mybir.dt.float32
BF16 = mybir.dt.bfloat16
EXP = mybir.ActivationFunctionType.Exp
LN = mybir.ActivationFunctionType.Ln
ADD = mybir.AluOpType.add
MULT = mybir.AluOpType.mult


def _emit(tc, nc, io, scale_val, bias_val):
    from contextlib import ExitStack

    ctx = ExitStack()
    with ctx:
        consts = ctx.enter_context(tc.tile_pool(name="consts", bufs=1))
        sb = ctx.enter_context(tc.tile_pool(name="sb", bufs=1))
        scr = ctx.enter_context(tc.tile_pool(name="scr", bufs=2))
        pP = ctx.enter_context(tc.tile_pool(name="pP", bufs=4))
        psU = ctx.enter_context(tc.tile_pool(name="psU", bufs=2, space="PSUM"))
        psPV = ctx.enter_context(tc.tile_pool(name="psPV", bufs=1, space="PSUM"))
        psS = ctx.enter_context(tc.tile_pool(name="psS", bufs=2, space="PSUM"))

        # ---- constants / weights (Q-path inputs first so Q proj starts early)
        ident2 = consts.tile([P, 64], F32)
        nc.sync.dma_start(ident2[:], io["ident2"].ap())
        # mask32[:, j, :]: lift-mask variant writing head-sums to rows {2j,2j+1}
        mask32 = consts.tile([P, 16, 32], BF16)
        nc.sync.dma_start(mask32[:], io["mask32"].ap())

        w_sb = {}
        b_sb = {}

        def load_w(nm):
            w = consts.tile([P, 4, 4, P], BF16, name=f"{nm}_sb")
            nc.sync.dma_start(w[:], io[nm].ap())
            w_sb[nm] = w
            bn = "b" + nm[1]
            bt = consts.tile([P, 4], F32, name=f"{bn}_sb")
            nc.sync.dma_start(bt[:], io[bn].ap())
            b_sb[bn] = bt

        load_w("wq")
        xq = sb.tile([P, 4, QB], BF16)
        nc.sync.dma_start(xq[:], io["xq_t"].ap())

        ebias = consts.tile([P, 1], F32)
        nc.vector.memset(ebias[:], 2.0 / scale_val + bias_val)

        qsT = sb.tile([P, NHP, QB], BF16)
        ksT = sb.tile([P, NHP, N], BF16)
        vT = sb.tile([P, NHP, N], BF16)
        v_nat = sb.tile([P, 16, NHP, P], BF16)  # [p, mc, hp, 2h*64]; key=mc*128+p

        def project(dst_sl, x_sl, w, pt, bias, qcs):
            for qc in qcs:
                ps = psU.tile([P, 512], F32, tag="u", name="proj")
                for ec in range(4):
                    nc.tensor.matmul(
                        ps[:],
                        lhsT=w[:, ec, pt, :],
                        rhs=x_sl[:, ec, qc * 512 : (qc + 1) * 512],
                        start=(ec == 0),
                        stop=(ec == 3),
                    )
                nc.vector.tensor_tensor(
                    dst_sl[:, qc * 512 : (qc + 1) * 512],
                    ps[:],
                    bias.to_broadcast((P, 512)),
                    ADD,
                )

        def sqrt_via_lnexp(dst, src, ln_scale=1.0, ln_bias=0.0, exp_scale=0.5):
            """dst = exp(exp_scale * ln(ln_scale*src + ln_bias))."""
            lg = scr.tile(list(src.shape), F32, tag="lg", name="lg")
            nc.scalar.activation(lg[:], src, LN, bias=ln_bias, scale=ln_scale)
            nc.scalar.activation(dst, lg[:], EXP, scale=exp_scale)

        # ---- Q projection + lift (all 4 head-pairs) ----
        for hp in range(NHP):
            project(
                qsT[:, hp, :], xq, w_sb["wq"], hp, b_sb["bq"][:, hp : hp + 1], [0]
            )
        qsq = sb.tile([P, NHP, QB], BF16)
        nc.vector.tensor_tensor(qsq[:], qsT[:], qsT[:], MULT)
        qnrm = psU.tile([8, 512], F32, tag="u", name="qnrm")
        for hp in range(NHP):
            nc.tensor.matmul(
                qnrm[:],
                lhsT=mask32[:, hp, 0:8],
                rhs=qsq[:, hp, :],
                start=(hp == 0),
                stop=(hp == NHP - 1),
            )
        qt_s = scr.tile([8, 512], BF16, tag="qts", bufs=1)
        sqrt_via_lnexp(qt_s[:], qnrm[:], ln_bias=1.0)
        for hp in range(NHP):
            nc.sync.dma_start(qsT[0:65:64, hp, :], qt_s[2 * hp : 2 * hp + 2, :])

        # source-side inputs (after the Q chain so xq lands first); xs arrives
        # in 4 column chunks so the first K projection starts ASAP.
        load_w("wk")
        load_w("wv")
        xs = sb.tile([P, 4, N], BF16)
        for qc in range(4):
            nc.sync.dma_start(
                xs[:, :, qc * 512 : (qc + 1) * 512], io[f"xs{qc}"].ap()
            )

        act_scale = -2.0 / scale_val
        pv_tiles = {}

        def prologue_hp(hp):
            """Project + lift K and V for head-pair hp, then transpose V."""
            project(ksT[:, hp, :], xs, w_sb["wk"], hp, b_sb["bk"][:, hp : hp + 1], range(4))
            project(vT[:, hp, :], xs, w_sb["wv"], hp, b_sb["bv"][:, hp : hp + 1], range(4))
            # 16 time^2 rows (2 proj x 4 chunks x 2 heads) -> one PSUM bank via
            # accumulating mask-variant matmuls (each adds 2 rows + zeros).
            kvnrm = psU.tile([16, 512], F32, tag="u", name="kvnrm")
            nmm = 0
            for pi, src in enumerate((ksT, vT)):
                sq = scr.tile([P, N], BF16, tag="ksq")
                nc.vector.tensor_tensor(sq[:], src[:, hp, :], src[:, hp, :], MULT)
                for qc in range(4):
                    nc.tensor.matmul(
                        kvnrm[:],
                        lhsT=mask32[:, 4 * pi + qc, 0:16],
                        rhs=sq[:, qc * 512 : (qc + 1) * 512],
                        start=(nmm == 0),
                        stop=(nmm == 7),
                    )
                    nmm += 1
            kvt = scr.tile([16, 512], BF16, tag="kvt")
            sqrt_via_lnexp(kvt[:], kvnrm[:], ln_bias=1.0)
            for pi, dst in enumerate((ksT, vT)):
                for qc in range(4):
                    r = 8 * pi + 2 * qc
                    nc.sync.dma_start(
                        dst[0:65:64, hp, qc * 512 : (qc + 1) * 512],
                        kvt[r : r + 2, :],
                    )
            # V -> natural layout in ONE xbar transpose:
            # transposed row r (= key) lands at v_nat[r%128, r//128, hp, :].
            nc.sync.dma_start(v_nat[:, :, hp, :], vT[:, hp, :], transpose=True)

        def attention_hp(hp):
            pv_tiles[hp] = psPV.tile([P, QB], F32, name=f"pv{hp}", tag=f"pv{hp % 2}")
            for mc in range(16):
                s_ps = psS.tile([P, 1024], F32, tag="s")
                for h in range(2):
                    nc.tensor.matmul(
                        s_ps[:, h * 512 : (h + 1) * 512],
                        lhsT=ksT[h * 64 : (h + 1) * 64, hp, mc * P : (mc + 1) * P],
                        rhs=qsT[h * 64 : (h + 1) * 64, hp, :],
                        start=True,
                        stop=True,
                    )
                p_sb = pP.tile([P, 1024], BF16, tag="p")
                nc.scalar.activation(
                    p_sb[:], s_ps[:], EXP, scale=act_scale, bias=ebias[:]
                )
                for h in range(2):
                    nc.tensor.matmul(
                        pv_tiles[hp][h * 64 : (h + 1) * 64, :],
                        lhsT=v_nat[:, mc, hp, h * 64 : (h + 1) * 64],
                        rhs=p_sb[:, h * 512 : (h + 1) * 512],
                        start=(mc == 0),
                        stop=(mc == 15),
                        skip_group_check=True,
                    )

        o_unT = sb.tile([P, NHP, QB], F32)
        o_nat = sb.tile([P, 4, H, D], F32)  # [q%128, qtile, head, d]
        ph_sq = sb.tile([P, 4, H, D], F32)
        ph_inner = sb.tile([P, 4, H, 1], F32)
        ph_t2 = sb.tile([P, 4, H, 1], F32)

        def tail_hp(hp):
            """Drain PV, transpose to natural layout, DVE part of the per-head
            centroid (no ACT ops -> EXP stream stays uninterrupted)."""
            nc.vector.tensor_copy(out=o_unT[:, hp, :], in_=pv_tiles[hp][:])
            for hh in range(2):
                h = 2 * hp + hh
                for qt in range(4):
                    pt = psU.tile([P, 64], F32, tag="u", name="otp")
                    nc.tensor.transpose(
                        pt[:],
                        o_unT[hh * 64 : (hh + 1) * 64, hp, qt * P : (qt + 1) * P],
                        ident2[hh * 64 : (hh + 1) * 64, :],
                    )
                    nc.vector.tensor_copy(out=o_nat[:, qt, h, :], in_=pt[:])
            hsl = slice(2 * hp, 2 * hp + 2)
            src = o_nat[:, :, hsl, :]
            nc.vector.tensor_tensor(ph_sq[:, :, hsl, :], src, src, MULT)
            nc.vector.tensor_reduce(
                ph_inner[:, :, hsl, 0],
                ph_sq[:, :, hsl, :],
                axis=mybir.AxisListType.X,
                op=ADD,
            )
            nc.vector.tensor_tensor(
                ph_t2[:, :, hsl, :], src[:, :, :, 0:1], src[:, :, :, 0:1], MULT
            )
            nc.vector.tensor_scalar_mul(ph_t2[:, :, hsl, :], ph_t2[:, :, hsl, :], -2.0)
            nc.vector.tensor_tensor(
                ph_inner[:, :, hsl, :], ph_inner[:, :, hsl, :], ph_t2[:, :, hsl, :], ADD
            )

        for hp in range(NHP):
            prologue_hp(hp)
            attention_hp(hp)
            if hp > 0:
                tail_hp(hp - 1)  # overlaps attention_hp(hp)
        tail_hp(NHP - 1)

        # ---- per-head centroid scale, head-sum, final centroid ----
        rec = sb.tile([P, 4, H, 1], F32)
        sqrt_via_lnexp(rec[:], ph_inner[:], ln_scale=-1.0, exp_scale=-0.5)
        cent = sb.tile([P, 4, H, D], F32)
        nc.vector.tensor_tensor(
            cent[:], o_nat[:], rec[:].to_broadcast((P, 4, H, D)), MULT
        )
        # head-sum as a contiguous binary tree (strided reduce is slow on DVE)
        c4 = cent[:]  # [P, 4, 8, 64]
        h4 = sb.tile([P, 4, 4, D], F32)
        nc.vector.tensor_tensor(h4[:], c4[:, :, 0:4, :], c4[:, :, 4:8, :], ADD)
        h2 = sb.tile([P, 4, 2, D], F32)
        nc.vector.tensor_tensor(h2[:], h4[:, :, 0:2, :], h4[:, :, 2:4, :], ADD)
        hsum = sb.tile([P, 4, 1, D], F32)
        nc.vector.tensor_tensor(hsum[:], h2[:, :, 0:1, :], h2[:, :, 1:2, :], ADD)
        fsq = sb.tile([P, 4, 1, D], F32)
        nc.vector.tensor_tensor(fsq[:], hsum[:], hsum[:], MULT)
        finner = sb.tile([P, 4, 1, 1], F32)
        nc.vector.tensor_reduce(
            finner[:, :, :, 0], fsq[:], axis=mybir.AxisListType.X, op=ADD
        )
        ft2 = sb.tile([P, 4, 1, 1], F32)
        nc.vector.tensor_tensor(ft2[:], hsum[:, :, :, 0:1], hsum[:, :, :, 0:1], MULT)
        nc.vector.tensor_scalar_mul(ft2[:], ft2[:], -2.0)
        nc.vector.tensor_tensor(finner[:], finner[:], ft2[:], ADD)
        frec = sb.tile([P, 4, 1, 1], F32)
        sqrt_via_lnexp(frec[:], finner[:], ln_scale=-1.0, exp_scale=-0.5)
        out_sb = sb.tile([P, 4, D], F32)
        nc.vector.tensor_tensor(
            out_sb[:],
            hsum[:, :, 0, :],
            frec[:, :, 0, :].to_broadcast((P, 4, D)),
            MULT,
        )
        nc.sync.dma_start(
            io["out"].ap().rearrange("(t p) d -> p t d", p=P), out_sb[:]
        )


def _build(scale_val, bias_val):
    nc = bacc.Bacc(num_devices=N_CORES)
    io = {}
    io["xq_t"] = nc.declare_dram_parameter("xq_t", [P, 4, QB], BF16, isOutput=False)
    for qc in range(4):
        io[f"xs{qc}"] = nc.declare_dram_parameter(
            f"xs{qc}", [P, 4, 512], BF16, isOutput=False
        )
    for nm in ("wq", "wk", "wv"):
        io[nm] = nc.declare_dram_parameter(nm, [P, 4, 4, P], BF16, isOutput=False)
    for nm in ("bq", "bk", "bv"):
        io[nm] = nc.declare_dram_parameter(nm, [P, 4], F32, isOutput=False)
    io["ident2"] = nc.declare_dram_parameter("ident2", [P, 64], F32, isOutput=False)
    io["mask32"] = nc.declare_dram_parameter("mask32", [P, 16, 32], BF16, isOutput=False)
    io["out"] = nc.declare_dram_parameter("out", [QB, D], F32, isOutput=True)

    with tile.TileContext(nc) as tc:
        _emit(tc, nc, io, scale_val, bias_val)
    nc.compile()
    return nc


_BUILD_CACHE = {}


def _get_nc(scale_val, bias_val):
    key = (float(scale_val), float(bias_val))
    if key not in _BUILD_CACHE:
        _BUILD_CACHE[key] = _build(*key)
    return _BUILD_CACHE[key]


def _pad_wT8(w):
    """w: [504, 512] spatial weights for 8 heads -> [512, 512] transposed with
    zero columns at each head's time slot (col h*64)."""
    out = np.zeros((E, 512), dtype=np.float32)
    for h in range(H):
        out[:, h * 64 + 1 : (h + 1) * 64] = w[h * DM1 : (h + 1) * DM1, :].T
    return np.ascontiguousarray(out)


def _pad_b8(b):
    out = np.zeros((512,), dtype=np.float32)
    for h in range(H):
        out[h * 64 + 1 : (h + 1) * 64] = b[h * DM1 : (h + 1) * DM1]
    return out


def _fmt_w(wpad, BF):
    # [E, 512] -> [128 p, 4 ec, 4 pt, 128 m]
    return np.ascontiguousarray(
        wpad.reshape(4, P, 4, P).transpose(1, 0, 2, 3)
    ).astype(BF)


def _fmt_x(x_t, BF):
    # [E, ncols] -> [128 p, 4 ec, ncols]
    return np.ascontiguousarray(
        x_t.reshape(4, P, x_t.shape[1]).transpose(1, 0, 2)
    ).astype(BF)


def make_in_maps(
    query_input, source_input, Wq_w, Wq_b, Wk_w, Wk_b, Wv_w, Wv_b, scale, bias
):
    import ml_dtypes

    BF = ml_dtypes.bfloat16
    ident2 = np.concatenate([np.eye(64), np.eye(64)], axis=0).astype(np.float32)
    mask32 = np.zeros((P, 16, 32), dtype=np.float32)
    for j in range(16):
        mask32[1:64, j, 2 * j] = 1.0
        mask32[65:128, j, 2 * j + 1] = 1.0
    mask32 = mask32.astype(BF)

    wq = _fmt_w(_pad_wT8(Wq_w), BF)
    wk = _fmt_w(_pad_wT8(-Wk_w), BF)  # Lorentz sign folded into K
    wv = _fmt_w(_pad_wT8(Wv_w), BF)
    bq = np.ascontiguousarray(_pad_b8(Wq_b).reshape(4, P).T)
    bk = np.ascontiguousarray(_pad_b8(-Wk_b).reshape(4, P).T)
    bv = np.ascontiguousarray(_pad_b8(Wv_b).reshape(4, P).T)

    xs_chunks = []
    for b in range(B):
        xt = source_input[b].T  # [E, N]
        xs_chunks.append(
            [_fmt_x(xt[:, qc * 512 : (qc + 1) * 512], BF) for qc in range(4)]
        )

    in_maps = []
    for c in range(N_CORES):
        b = c // 4
        g = c % 4
        m = {
            "xq_t": _fmt_x(query_input[b, g * QB : (g + 1) * QB, :].T, BF),
            "wq": wq,
            "wk": wk,
            "wv": wv,
            "bq": bq,
            "bk": bk,
            "bv": bv,
            "ident2": ident2,
            "mask32": mask32,
        }
        for qc in range(4):
            m[f"xs{qc}"] = xs_chunks[b][qc]
        in_maps.append(m)
    return in_maps


def kernel(
    query_input,
    source_input,
    Wq_w,
    Wq_b,
    Wk_w,
    Wk_b,
    Wv_w,
    Wv_b,
    scale,
    bias,
    _trace=False,
):
    scale_val = float(np.asarray(scale).reshape(-1)[0])
    bias_val = float(np.asarray(bias).reshape(-1)[0]) if np.asarray(bias).size else 0.0

    nc = _get_nc(scale_val, bias_val)
    in_maps = make_in_maps(
        query_input, source_input, Wq_w, Wq_b, Wk_w, Wk_b, Wv_w, Wv_b, scale, bias
    )

    from concourse.bass_utils import run_bass_kernel_spmd

    res = run_bass_kernel_spmd(
        nc, in_maps, core_ids=list(range(N_CORES)), trace=_trace
    )

    out = np.zeros((B, N, D), dtype=np.float32)
    for c in range(N_CORES):
        b = c // 4
        g = c % 4
        out[b, g * QB : (g + 1) * QB, :] = res.results[c]["out"]
    if _trace:
        kernel.last_exec_time_ns = res.exec_time_ns
        kernel.last_results = res
    return out


# revision 25
# speedup vs baseline: 1.5943x; 1.0200x over previous
"""Trainium2 Bass kernel for LorentzMultiheadAttention (B=2, N=2048, H=8, D=64, E=512).

Sharding: 8 cores = 2 batches x 4 query-quarters. Core c handles batch b=c//4
and queries [512*(c%4), 512*(c%4+1)) for ALL 8 heads. K/V projections are
recomputed on each core of a batch group (cheaper than an inter-core
ReduceScatter) so the kernel has NO collectives: per-head centroids, the head
mean, and the second centroid are all core-local.

Structure: a 4-stage pipeline over head-pairs. For each head-pair hp:
project K/V -> lift -> one xbar-transpose of V -> 16 attention iterations
(2 score MMs -> EXP[128,1024] -> 2 PV MMs). The next head-pair's projections
are scheduled into the PE slack under the current pair's (ACT-bound) EXP
stream. PSUM: 4 score banks + 2 PV banks (alternating per hp) + 2 utility
banks (projections / lift sums / output transposes) = 8.

ACT table-set discipline: sqrt(x) is computed as exp(0.5*ln(x)) and
1/sqrt(|x|) as exp(-0.5*ln(-x)) so every ACT instruction (lifts, attention
EXPs, centroid scales) draws from the natural_log_exp table set -- no
mid-stream ACT_TABLE_LOADs.

Layout tricks:
- The [128,2048]->[128,16,128] DMA xbar transpose delivers transposed row r
  (= key index) to partition r%128, free tile r//128 -- exactly the
  natural-layout V that PV matmuls need, in ONE instruction per head-pair.
- Lift time rows: accumulating mask-variant matmuls pack all of a head-pair's
  sum(x_spatial^2) rows (K and V, 4 column chunks, even/odd head) into one
  PSUM bank at rows 0..15, so a single 16-lane Ln+Exp computes every
  t = sqrt(1+s); small SBUF->SBUF DMAs scatter the two time rows of each
  target to partitions {0,64}.

Math notes:
- The Lorentz centroid sqrt(C)*x/sqrt(|<x,x>_L|) is scale-invariant, so the
  softmax denominator and the mean-over-heads divide both cancel; PV feeds
  unnormalized sum_m exp(att)*v into the centroid.
- The Lorentz sign is folded by negating K weights on the host:
  scores S' = t_q*t_k - q_s.k_s = -<q,k>_L and softmax weights are
  exp(-(2/s)*S' + (2/s + bias)). No max-subtraction: |att| <= ~3.
"""

import os
import sys

for _p in ("/opt/trn_rl_repo", "/root/.axon_site/_ro/trn_rl_repo"):
    if os.path.isdir(_p) and _p not in sys.path:
        sys.path.insert(0, _p)

import numpy as np

import concourse.bacc as bacc
import concourse.bass as bass
import concourse.mybir as mybir
import concourse.tile as tile

B = 2
N = 2048
H = 8
D = 64
E = 512
DM1 = D - 1  # 63
P = 128
N_CORES = 8
QB = N // 4  # 512 queries per core
NHP = 4  # head-pairs per core

F32 = mybir.dt.float32
BF16 = mybir.dt.bfloat16
I32 = mybir.dt.int32
EXP = mybir.ActivationFunctionType.Exp
SQRT = mybir.ActivationFunctionType.Sqrt
ADD = mybir.AluOpType.add
SUB = mybir.AluOpType.subtract
MULT = mybir.AluOpType.mult
SHR = mybir.AluOpType.logical_shift_right
QMAGIC = 0x5F3759DF


def _emit(tc, nc, io, scale_val, bias_val):
    from contextlib import ExitStack

    ctx = ExitStack()
    with ctx:
        consts = ctx.enter_context(tc.tile_pool(name="consts", bufs=1))
        sb = ctx.enter_context(tc.tile_pool(name="sb", bufs=1))
        scr = ctx.enter_context(tc.tile_pool(name="scr", bufs=2))
        pP = ctx.enter_context(tc.tile_pool(name="pP", bufs=4))
        psU = ctx.enter_context(tc.tile_pool(name="psU", bufs=2, space="PSUM"))
        psPV = ctx.enter_context(tc.tile_pool(name="psPV", bufs=1, space="PSUM"))
        psS = ctx.enter_context(tc.tile_pool(name="psS", bufs=2, space="PSUM"))

        # ---- constants / weights (Q-path inputs first so Q proj starts early)
        ident2 = consts.tile([P, 64], F32)
        nc.sync.dma_start(ident2[:], io["ident2"].ap())
        # mask32[:, j, :]: lift-mask variant writing head-sums to rows {2j,2j+1}
        mask32 = consts.tile([P, 16, 32], BF16)
        nc.sync.dma_start(mask32[:], io["mask32"].ap())

        w_sb = {}
        b_sb = {}

        def load_w(nm):
            w = consts.tile([P, 4, 4, P], BF16, name=f"{nm}_sb")
            nc.sync.dma_start(w[:], io[nm].ap())
            w_sb[nm] = w
            bn = "b" + nm[1]
            bt = consts.tile([P, 4], F32, name=f"{bn}_sb")
            nc.sync.dma_start(bt[:], io[bn].ap())
            b_sb[bn] = bt

        load_w("wq")
        xq = sb.tile([P, 4, QB], BF16)
        nc.sync.dma_start(xq[:], io["xq_t"].ap())

        ebias = consts.tile([P, 1], F32)
        nc.vector.memset(ebias[:], 2.0 / scale_val + bias_val)

        qsT = sb.tile([P, NHP, QB], BF16)
        ksT = sb.tile([P, NHP, N], BF16)
        vT = sb.tile([P, NHP, N], BF16)
        v_nat = sb.tile([P, 16, NHP, P], BF16)  # [p, mc, hp, 2h*64]; key=mc*128+p

        def project(dst_sl, x_sl, w, pt, bias, qcs):
            for qc in qcs:
                ps = psU.tile([P, 512], F32, tag="u", name="proj")
                for ec in range(4):
                    nc.tensor.matmul(
                        ps[:],
                        lhsT=w[:, ec, pt, :],
                        rhs=x_sl[:, ec, qc * 512 : (qc + 1) * 512],
                        start=(ec == 0),
                        stop=(ec == 3),
                    )
                nc.vector.tensor_tensor(
                    dst_sl[:, qc * 512 : (qc + 1) * 512],
                    ps[:],
                    bias.to_broadcast((P, 512)),
                    ADD,
                )

        qmagic = consts.tile([P, 1], I32)
        nc.vector.memset(qmagic[:], QMAGIC)

        def rsqrt_dve(u, tag, iters=1):
            """1/sqrt(u) on the vector engine: Quake seed + Newton steps.
            u: f32 SBUF AP. Returns an f32 tile of the same shape. Keeps the
            ACT engine (and its exp-table) untouched by the lifts."""
            shp = list(u.shape)
            y = scr.tile(shp, F32, tag=f"{tag}y", name="qk_y")
            sh = scr.tile(shp, I32, tag=f"{tag}i", name="qk_i")
            nc.vector.tensor_scalar(sh[:], u.bitcast(I32), 1, None, SHR)
            nc.vector.tensor_tensor(
                y[:].bitcast(I32),
                qmagic[0 : shp[0], :].to_broadcast(tuple(shp)),
                sh[:],
                SUB,
            )
            z = scr.tile(shp, F32, tag=f"{tag}z", name="qk_z")
            for _ in range(iters):
                nc.vector.tensor_tensor(z[:], y[:], y[:], MULT)
                nc.vector.tensor_tensor(z[:], u, z[:], MULT)
                nc.vector.tensor_scalar(z[:], z[:], -0.5, 1.5, MULT, ADD)
                nc.vector.tensor_tensor(y[:], y[:], z[:], MULT)
            return y

        def lift_times(dst, nrm_ps, tag):
            """dst (bf16) = sqrt(1 + nrm_ps) via u*rsqrt(u), DVE-only."""
            shp = list(nrm_ps.shape)
            u = scr.tile(shp, F32, tag=f"{tag}u", name="qk_u")
            nc.vector.tensor_scalar(u[:], nrm_ps, 1.0, None, ADD)
            y = rsqrt_dve(u[:], tag)
            nc.vector.tensor_tensor(dst, u[:], y[:], MULT)

        # ---- Q projection + lift (all 4 head-pairs) ----
        for hp in range(NHP):
            project(
                qsT[:, hp, :], xq, w_sb["wq"], hp, b_sb["bq"][:, hp : hp + 1], [0]
            )
        qsq = sb.tile([P, NHP, QB], BF16)
        nc.vector.tensor_tensor(qsq[:], qsT[:], qsT[:], MULT)
        qnrm = psU.tile([8, 512], F32, tag="u", name="qnrm")
        for hp in range(NHP):
            nc.tensor.matmul(
                qnrm[:],
                lhsT=mask32[:, hp, 0:8],
                rhs=qsq[:, hp, :],
                start=(hp == 0),
                stop=(hp == NHP - 1),
            )
        qt_s = scr.tile([8, 512], BF16, tag="qts", bufs=1)
        lift_times(qt_s[:], qnrm[:], "qk8")
        for hp in range(NHP):
            nc.sync.dma_start(qsT[0:65:64, hp, :], qt_s[2 * hp : 2 * hp + 2, :])

        # source-side inputs (after the Q chain so xq lands first); xs arrives
        # in 4 column chunks so the first K projection starts ASAP.
        load_w("wk")
        load_w("wv")
        xs = sb.tile([P, 4, N], BF16)
        for qc in range(4):
            nc.sync.dma_start(
                xs[:, :, qc * 512 : (qc + 1) * 512], io[f"xs{qc}"].ap()
            )

        act_scale = -2.0 / scale_val
        pv_tiles = {}

        def prologue_hp(hp):
            """Project + lift K and V for head-pair hp, then transpose V."""
            project(ksT[:, hp, :], xs, w_sb["wk"], hp, b_sb["bk"][:, hp : hp + 1], range(4))
            project(vT[:, hp, :], xs, w_sb["wv"], hp, b_sb["bv"][:, hp : hp + 1], range(4))
            # 16 time^2 rows (2 proj x 4 chunks x 2 heads) -> one PSUM bank via
            # accumulating mask-variant matmuls (each adds 2 rows + zeros).
            kvnrm = psU.tile([16, 512], F32, tag="u", name="kvnrm")
            nmm = 0
            for pi, src in enumerate((ksT, vT)):
                sq = scr.tile([P, N], BF16, tag="ksq")
                nc.vector.tensor_tensor(sq[:], src[:, hp, :], src[:, hp, :], MULT)
                for qc in range(4):
                    nc.tensor.matmul(
                        kvnrm[:],
                        lhsT=mask32[:, 4 * pi + qc, 0:16],
                        rhs=sq[:, qc * 512 : (qc + 1) * 512],
                        start=(nmm == 0),
                        stop=(nmm == 7),
                    )
                    nmm += 1
            kvt = scr.tile([16, 512], BF16, tag="kvt")
            lift_times(kvt[:], kvnrm[:], "qk16")
            for pi, dst in enumerate((ksT, vT)):
                for qc in range(4):
                    r = 8 * pi + 2 * qc
                    nc.sync.dma_start(
                        dst[0:65:64, hp, qc * 512 : (qc + 1) * 512],
                        kvt[r : r + 2, :],
                    )
            # V -> natural layout in ONE xbar transpose:
            # transposed row r (= key) lands at v_nat[r%128, r//128, hp, :].
            nc.sync.dma_start(v_nat[:, :, hp, :], vT[:, hp, :], transpose=True)

        def attention_hp(hp):
            pv_tiles[hp] = psPV.tile([P, QB], F32, name=f"pv{hp}", tag=f"pv{hp % 2}")
            for mc in range(16):
                s_ps = psS.tile([P, 1024], F32, tag="s")
                for h in range(2):
                    nc.tensor.matmul(
                        s_ps[:, h * 512 : (h + 1) * 512],
                        lhsT=ksT[h * 64 : (h + 1) * 64, hp, mc * P : (mc + 1) * P],
                        rhs=qsT[h * 64 : (h + 1) * 64, hp, :],
                        start=True,
                        stop=True,
                    )
                p_sb = pP.tile([P, 1024], BF16, tag="p")
                nc.scalar.activation(
                    p_sb[:], s_ps[:], EXP, scale=act_scale, bias=ebias[:]
                )
                for h in range(2):
                    nc.tensor.matmul(
                        pv_tiles[hp][h * 64 : (h + 1) * 64, :],
                        lhsT=v_nat[:, mc, hp, h * 64 : (h + 1) * 64],
                        rhs=p_sb[:, h * 512 : (h + 1) * 512],
                        start=(mc == 0),
                        stop=(mc == 15),
                        skip_group_check=True,
                    )

        o_unT = sb.tile([P, NHP, QB], F32)
        o_nat = sb.tile([P, 4, H, D], F32)  # [q%128, qtile, head, d]
        ph_sq = sb.tile([P, 4, H, D], F32)
        ph_inner = sb.tile([P, 4, H, 1], F32)
        ph_t2 = sb.tile([P, 4, H, 1], F32)

        def tail_hp(hp):
            """Drain PV, transpose to natural layout, DVE part of the per-head
            centroid (no ACT ops -> EXP stream stays uninterrupted)."""
            nc.vector.tensor_copy(out=o_unT[:, hp, :], in_=pv_tiles[hp][:])
            for hh in range(2):
                h = 2 * hp + hh
                for qt in range(4):
                    pt = psU.tile([P, 64], F32, tag="u", name="otp")
                    nc.tensor.transpose(
                        pt[:],
                        o_unT[hh * 64 : (hh + 1) * 64, hp, qt * P : (qt + 1) * P],
                        ident2[hh * 64 : (hh + 1) * 64, :],
                    )
                    nc.vector.tensor_copy(out=o_nat[:, qt, h, :], in_=pt[:])
            hsl = slice(2 * hp, 2 * hp + 2)
            src = o_nat[:, :, hsl, :]
            nc.vector.tensor_tensor(ph_sq[:, :, hsl, :], src, src, MULT)
            nc.vector.tensor_reduce(
                ph_inner[:, :, hsl, 0],
                ph_sq[:, :, hsl, :],
                axis=mybir.AxisListType.X,
                op=ADD,
            )
            nc.vector.tensor_tensor(
                ph_t2[:, :, hsl, :], src[:, :, :, 0:1], src[:, :, :, 0:1], MULT
            )
            nc.vector.tensor_scalar_mul(ph_t2[:, :, hsl, :], ph_t2[:, :, hsl, :], -2.0)
            nc.vector.tensor_tensor(
                ph_inner[:, :, hsl, :], ph_inner[:, :, hsl, :], ph_t2[:, :, hsl, :], ADD
            )

        for hp in range(NHP):
            prologue_hp(hp)
            attention_hp(hp)
            if hp > 0:
                tail_hp(hp - 1)  # overlaps attention_hp(hp)
        tail_hp(NHP - 1)

        # ---- per-head centroid scale, head-sum, final centroid ----
        den = sb.tile([P, 4, H, 1], F32)
        nc.scalar.activation(den[:], ph_inner[:], SQRT, bias=0.0, scale=-1.0)
        rec = sb.tile([P, 4, H, 1], F32)
        nc.vector.reciprocal(rec[:], den[:])
        cent = sb.tile([P, 4, H, D], F32)
        nc.vector.tensor_tensor(
            cent[:], o_nat[:], rec[:].to_broadcast((P, 4, H, D)), MULT
        )
        # head-sum as a contiguous binary tree (strided reduce is slow on DVE)
        c4 = cent[:]  # [P, 4, 8, 64]
        h4 = sb.tile([P, 4, 4, D], F32)
        nc.vector.tensor_tensor(h4[:], c4[:, :, 0:4, :], c4[:, :, 4:8, :], ADD)
        h2 = sb.tile([P, 4, 2, D], F32)
        nc.vector.tensor_tensor(h2[:], h4[:, :, 0:2, :], h4[:, :, 2:4, :], ADD)
        hsum = sb.tile([P, 4, 1, D], F32)
        nc.vector.tensor_tensor(hsum[:], h2[:, :, 0:1, :], h2[:, :, 1:2, :], ADD)
        fsq = sb.tile([P, 4, 1, D], F32)
        nc.vector.tensor_tensor(fsq[:], hsum[:], hsum[:], MULT)
        finner = sb.tile([P, 4, 1, 1], F32)
        nc.vector.tensor_reduce(
            finner[:, :, :, 0], fsq[:], axis=mybir.AxisListType.X, op=ADD
        )
        ft2 = sb.tile([P, 4, 1, 1], F32)
        nc.vector.tensor_tensor(ft2[:], hsum[:, :, :, 0:1], hsum[:, :, :, 0:1], MULT)
        nc.vector.tensor_scalar_mul(ft2[:], ft2[:], -2.0)
        nc.vector.tensor_tensor(finner[:], finner[:], ft2[:], ADD)
        fden = sb.tile([P, 4, 1, 1], F32)
        nc.scalar.activation(fden[:], finner[:], SQRT, bias=0.0, scale=-1.0)
        frec = sb.tile([P, 4, 1, 1], F32)
        nc.vector.reciprocal(frec[:], fden[:])
        out_sb = sb.tile([P, 4, D], F32)
        nc.vector.tensor_tensor(
            out_sb[:],
            hsum[:, :, 0, :],
            frec[:, :, 0, :].to_broadcast((P, 4, D)),
            MULT,
        )
        nc.sync.dma_start(
            io["out"].ap().rearrange("(t p) d -> p t d", p=P), out_sb[:]
        )


def _build(scale_val, bias_val):
    nc = bacc.Bacc(num_devices=N_CORES)
    io = {}
    io["xq_t"] = nc.declare_dram_parameter("xq_t", [P, 4, QB], BF16, isOutput=False)
    for qc in range(4):
        io[f"xs{qc}"] = nc.declare_dram_parameter(
            f"xs{qc}", [P, 4, 512], BF16, isOutput=False
        )
    for nm in ("wq", "wk", "wv"):
        io[nm] = nc.declare_dram_parameter(nm, [P, 4, 4, P], BF16, isOutput=False)
    for nm in ("bq", "bk", "bv"):
        io[nm] = nc.declare_dram_parameter(nm, [P, 4], F32, isOutput=False)
    io["ident2"] = nc.declare_dram_parameter("ident2", [P, 64], F32, isOutput=False)
    io["mask32"] = nc.declare_dram_parameter("mask32", [P, 16, 32], BF16, isOutput=False)
    io["out"] = nc.declare_dram_parameter("out", [QB, D], F32, isOutput=True)

    with tile.TileContext(nc) as tc:
        _emit(tc, nc, io, scale_val, bias_val)
    nc.compile()
    return nc


_BUILD_CACHE = {}


def _get_nc(scale_val, bias_val):
    key = (float(scale_val), float(bias_val))
    if key not in _BUILD_CACHE:
        _BUILD_CACHE[key] = _build(*key)
    return _BUILD_CACHE[key]


def _pad_wT8(w):
    """w: [504, 512] spatial weights for 8 heads -> [512, 512] transposed with
    zero columns at each head's time slot (col h*64)."""
    out = np.zeros((E, 512), dtype=np.float32)
    for h in range(H):
        out[:, h * 64 + 1 : (h + 1) * 64] = w[h * DM1 : (h + 1) * DM1, :].T
    return np.ascontiguousarray(out)


def _pad_b8(b):
    out = np.zeros((512,), dtype=np.float32)
    for h in range(H):
        out[h * 64 + 1 : (h + 1) * 64] = b[h * DM1 : (h + 1) * DM1]
    return out


def _fmt_w(wpad, BF):
    # [E, 512] -> [128 p, 4 ec, 4 pt, 128 m]
    return np.ascontiguousarray(
        wpad.reshape(4, P, 4, P).transpose(1, 0, 2, 3)
    ).astype(BF)


def _fmt_x(x_t, BF):
    # [E, ncols] -> [128 p, 4 ec, ncols]
    return np.ascontiguousarray(
        x_t.reshape(4, P, x_t.shape[1]).transpose(1, 0, 2)
    ).astype(BF)


def make_in_maps(
    query_input, source_input, Wq_w, Wq_b, Wk_w, Wk_b, Wv_w, Wv_b, scale, bias
):
    import ml_dtypes

    BF = ml_dtypes.bfloat16
    ident2 = np.concatenate([np.eye(64), np.eye(64)], axis=0).astype(np.float32)
    mask32 = np.zeros((P, 16, 32), dtype=np.float32)
    for j in range(16):
        mask32[1:64, j, 2 * j] = 1.0
        mask32[65:128, j, 2 * j + 1] = 1.0
    mask32 = mask32.astype(BF)

    wq = _fmt_w(_pad_wT8(Wq_w), BF)
    wk = _fmt_w(_pad_wT8(-Wk_w), BF)  # Lorentz sign folded into K
    wv = _fmt_w(_pad_wT8(Wv_w), BF)
    bq = np.ascontiguousarray(_pad_b8(Wq_b).reshape(4, P).T)
    bk = np.ascontiguousarray(_pad_b8(-Wk_b).reshape(4, P).T)
    bv = np.ascontiguousarray(_pad_b8(Wv_b).reshape(4, P).T)

    xs_chunks = []
    for b in range(B):
        xt = source_input[b].T  # [E, N]
        xs_chunks.append(
            [_fmt_x(xt[:, qc * 512 : (qc + 1) * 512], BF) for qc in range(4)]
        )

    in_maps = []
    for c in range(N_CORES):
        b = c // 4
        g = c % 4
        m = {
            "xq_t": _fmt_x(query_input[b, g * QB : (g + 1) * QB, :].T, BF),
            "wq": wq,
            "wk": wk,
            "wv": wv,
            "bq": bq,
            "bk": bk,
            "bv": bv,
            "ident2": ident2,
            "mask32": mask32,
        }
        for qc in range(4):
            m[f"xs{qc}"] = xs_chunks[b][qc]
        in_maps.append(m)
    return in_maps


def kernel(
    query_input,
    source_input,
    Wq_w,
    Wq_b,
    Wk_w,
    Wk_b,
    Wv_w,
    Wv_b,
    scale,
    bias,
    _trace=False,
):
    scale_val = float(np.asarray(scale).reshape(-1)[0])
    bias_val = float(np.asarray(bias).reshape(-1)[0]) if np.asarray(bias).size else 0.0

    nc = _get_nc(scale_val, bias_val)
    in_maps = make_in_maps(
        query_input, source_input, Wq_w, Wq_b, Wk_w, Wk_b, Wv_w, Wv_b, scale, bias
    )

    from concourse.bass_utils import run_bass_kernel_spmd

    res = run_bass_kernel_spmd(
        nc, in_maps, core_ids=list(range(N_CORES)), trace=_trace
    )

    out = np.zeros((B, N, D), dtype=np.float32)
    for c in range(N_CORES):
        b = c // 4
        g = c % 4
        out[b, g * QB : (g + 1) * QB, :] = res.results[c]["out"]
    if _trace:
        kernel.last_exec_time_ns = res.exec_time_ns
        kernel.last_results = res
    return out


# revision 26
# speedup vs baseline: 1.8086x; 1.1344x over previous
"""Trainium2 Bass kernel for LorentzMultiheadAttention (B=2, N=2048, H=8, D=64, E=512).

Sharding: 8 cores = 2 batches x 4 query-quarters. Core c handles batch b=c//4
and queries [512*(c%4), 512*(c%4+1)) for ALL 8 heads. K/V projections are
recomputed on each core of a batch group (cheaper than an inter-core
ReduceScatter) so the kernel has NO collectives: per-head centroids, the head
mean, and the second centroid are all core-local.

Structure: a 4-stage pipeline over head-pairs. For each head-pair hp:
project K/V -> lift -> one xbar-transpose of V -> 16 attention iterations
(2 score MMs -> EXP[128,1024] -> 2 PV MMs). The next head-pair's projections
are scheduled into the PE slack under the current pair's (ACT-bound) EXP
stream. PSUM: 4 score banks + 2 PV banks (alternating per hp) + 2 utility
banks (projections / lift sums / output transposes) = 8.

ACT table-set discipline: sqrt(x) is computed as exp(0.5*ln(x)) and
1/sqrt(|x|) as exp(-0.5*ln(-x)) so every ACT instruction (lifts, attention
EXPs, centroid scales) draws from the natural_log_exp table set -- no
mid-stream ACT_TABLE_LOADs.

Layout tricks:
- The [128,2048]->[128,16,128] DMA xbar transpose delivers transposed row r
  (= key index) to partition r%128, free tile r//128 -- exactly the
  natural-layout V that PV matmuls need, in ONE instruction per head-pair.
- Lift time rows: accumulating mask-variant matmuls pack all of a head-pair's
  sum(x_spatial^2) rows (K and V, 4 column chunks, even/odd head) into one
  PSUM bank at rows 0..15, so a single 16-lane Ln+Exp computes every
  t = sqrt(1+s); small SBUF->SBUF DMAs scatter the two time rows of each
  target to partitions {0,64}.

Math notes:
- The Lorentz centroid sqrt(C)*x/sqrt(|<x,x>_L|) is scale-invariant, so the
  softmax denominator and the mean-over-heads divide both cancel; PV feeds
  unnormalized sum_m exp(att)*v into the centroid.
- The Lorentz sign is folded by negating K weights on the host:
  scores S' = t_q*t_k - q_s.k_s = -<q,k>_L and softmax weights are
  exp(-(2/s)*S' + (2/s + bias)). No max-subtraction: |att| <= ~3.
"""

import os
import sys

for _p in ("/opt/trn_rl_repo", "/root/.axon_site/_ro/trn_rl_repo"):
    if os.path.isdir(_p) and _p not in sys.path:
        sys.path.insert(0, _p)

import numpy as np

import concourse.bacc as bacc
import concourse.bass as bass
import concourse.mybir as mybir
import concourse.tile as tile

B = 2
N = 2048
H = 8
D = 64
E = 512
DM1 = D - 1  # 63
P = 128
N_CORES = 8
QB = N // 4  # 512 queries per core
NHP = 4  # head-pairs per core

F32 = mybir.dt.float32
BF16 = mybir.dt.bfloat16
I32 = mybir.dt.int32
EXP = mybir.ActivationFunctionType.Exp
SQRT = mybir.ActivationFunctionType.Sqrt
ADD = mybir.AluOpType.add
SUB = mybir.AluOpType.subtract
MULT = mybir.AluOpType.mult
SHR = mybir.AluOpType.logical_shift_right
QMAGIC = 0x5F3759DF


def _emit(tc, nc, io, scale_val, bias_val):
    from contextlib import ExitStack

    ctx = ExitStack()
    with ctx:
        consts = ctx.enter_context(tc.tile_pool(name="consts", bufs=1))
        sb = ctx.enter_context(tc.tile_pool(name="sb", bufs=1))
        scr = ctx.enter_context(tc.tile_pool(name="scr", bufs=2))
        pP = ctx.enter_context(tc.tile_pool(name="pP", bufs=4))
        psU = ctx.enter_context(tc.tile_pool(name="psU", bufs=2, space="PSUM"))
        psPV = ctx.enter_context(tc.tile_pool(name="psPV", bufs=1, space="PSUM"))
        psS = ctx.enter_context(tc.tile_pool(name="psS", bufs=2, space="PSUM"))

        # ---- constants / weights (Q-path inputs first so Q proj starts early)
        ident2 = consts.tile([P, 64], F32)
        nc.sync.dma_start(ident2[:], io["ident2"].ap())
        # mask32[:, j, :]: lift-mask variant writing head-sums to rows {2j,2j+1}
        mask32 = consts.tile([P, 16, 32], BF16)
        nc.sync.dma_start(mask32[:], io["mask32"].ap())

        w_sb = {}
        b_sb = {}

        def load_w(nm):
            w = consts.tile([P, 4, 4, P], BF16, name=f"{nm}_sb")
            nc.sync.dma_start(w[:], io[nm].ap())
            w_sb[nm] = w
            bn = "b" + nm[1]
            bt = consts.tile([P, 4], F32, name=f"{bn}_sb")
            nc.sync.dma_start(bt[:], io[bn].ap())
            b_sb[bn] = bt

        load_w("wq")
        xq = sb.tile([P, 4, QB], BF16)
        nc.sync.dma_start(xq[:], io["xq_t"].ap())

        ebias = consts.tile([P, 1], F32)
        nc.vector.memset(ebias[:], 2.0 / scale_val + bias_val)

        qsT = sb.tile([P, NHP, QB], BF16)
        ksT = sb.tile([P, NHP, N], BF16)
        vT = sb.tile([P, NHP, N], BF16)
        v_nat = sb.tile([P, 16, NHP, P], BF16)  # [p, mc, hp, 2h*64]; key=mc*128+p

        IDENT = mybir.ActivationFunctionType.Identity

        def project(dst_sl, x_sl, w, pt, bias, qcs, copy_on_act=False):
            for qc in qcs:
                ps = psU.tile([P, 512], F32, tag="u", name="proj")
                for ec in range(4):
                    nc.tensor.matmul(
                        ps[:],
                        lhsT=w[:, ec, pt, :],
                        rhs=x_sl[:, ec, qc * 512 : (qc + 1) * 512],
                        start=(ec == 0),
                        stop=(ec == 3),
                    )
                dst = dst_sl[:, qc * 512 : (qc + 1) * 512]
                if copy_on_act:
                    # ACT is idle before the EXP stream starts; Identity is in
                    # every table set so this forces no ACT_TABLE_LOAD.
                    nc.scalar.activation(dst, ps[:], IDENT, bias=bias)
                else:
                    nc.vector.tensor_tensor(
                        dst, ps[:], bias.to_broadcast((P, 512)), ADD
                    )

        qmagic = consts.tile([P, 1], I32)
        nc.vector.memset(qmagic[:], QMAGIC)

        def rsqrt_dve(u, tag, iters=1):
            """1/sqrt(u) on the vector engine: Quake seed + Newton steps.
            u: f32 SBUF AP. Returns an f32 tile of the same shape. Keeps the
            ACT engine (and its exp-table) untouched by the lifts."""
            shp = list(u.shape)
            y = scr.tile(shp, F32, tag=f"{tag}y", name="qk_y")
            sh = scr.tile(shp, I32, tag=f"{tag}i", name="qk_i")
            nc.vector.tensor_scalar(sh[:], u.bitcast(I32), 1, None, SHR)
            nc.vector.tensor_tensor(
                y[:].bitcast(I32),
                qmagic[0 : shp[0], :].to_broadcast(tuple(shp)),
                sh[:],
                SUB,
            )
            z = scr.tile(shp, F32, tag=f"{tag}z", name="qk_z")
            for _ in range(iters):
                nc.vector.tensor_tensor(z[:], y[:], y[:], MULT)
                nc.vector.tensor_tensor(z[:], u, z[:], MULT)
                nc.vector.tensor_scalar(z[:], z[:], -0.5, 1.5, MULT, ADD)
                nc.vector.tensor_tensor(y[:], y[:], z[:], MULT)
            return y

        def lift_times(dst, nrm_ps, tag):
            """dst (bf16) = sqrt(1 + nrm_ps) via u*rsqrt(u), DVE-only."""
            shp = list(nrm_ps.shape)
            u = scr.tile(shp, F32, tag=f"{tag}u", name="qk_u")
            nc.vector.tensor_scalar(u[:], nrm_ps, 1.0, None, ADD)
            y = rsqrt_dve(u[:], tag)
            nc.vector.tensor_tensor(dst, u[:], y[:], MULT)

        # ---- Q projection + lift (all 4 head-pairs) ----
        for hp in range(NHP):
            project(
                qsT[:, hp, :], xq, w_sb["wq"], hp, b_sb["bq"][:, hp : hp + 1], [0],
                copy_on_act=True,
            )
        qsq = sb.tile([P, NHP, QB], BF16)
        nc.vector.tensor_tensor(qsq[:], qsT[:], qsT[:], MULT)
        qnrm = psU.tile([8, 512], F32, tag="u", name="qnrm")
        for hp in range(NHP):
            nc.tensor.matmul(
                qnrm[:],
                lhsT=mask32[:, hp, 0:8],
                rhs=qsq[:, hp, :],
                start=(hp == 0),
                stop=(hp == NHP - 1),
            )
        qt_s = scr.tile([8, 512], BF16, tag="qts", bufs=1)
        lift_times(qt_s[:], qnrm[:], "qk8")
        for hp in range(NHP):
            nc.sync.dma_start(qsT[0:65:64, hp, :], qt_s[2 * hp : 2 * hp + 2, :])

        # source-side inputs (after the Q chain so xq lands first); xs arrives
        # in 4 column chunks so the first K projection starts ASAP.
        load_w("wk")
        load_w("wv")
        xs = sb.tile([P, 4, N], BF16)
        for qc in range(4):
            nc.sync.dma_start(
                xs[:, :, qc * 512 : (qc + 1) * 512], io[f"xs{qc}"].ap()
            )

        act_scale = -2.0 / scale_val
        pv_tiles = {}

        def prologue_hp(hp):
            """Project + lift K and V for head-pair hp, then transpose V."""
            on_act = hp == 0
            project(ksT[:, hp, :], xs, w_sb["wk"], hp, b_sb["bk"][:, hp : hp + 1],
                    range(4), copy_on_act=on_act)
            project(vT[:, hp, :], xs, w_sb["wv"], hp, b_sb["bv"][:, hp : hp + 1],
                    range(4), copy_on_act=on_act)
            # 16 time^2 rows (2 proj x 4 chunks x 2 heads) -> one PSUM bank via
            # accumulating mask-variant matmuls (each adds 2 rows + zeros).
            kvnrm = psU.tile([16, 512], F32, tag="u", name="kvnrm")
            nmm = 0
            for pi, src in enumerate((ksT, vT)):
                sq = scr.tile([P, N], BF16, tag="ksq")
                nc.vector.tensor_tensor(sq[:], src[:, hp, :], src[:, hp, :], MULT)
                for qc in range(4):
                    nc.tensor.matmul(
                        kvnrm[:],
                        lhsT=mask32[:, 4 * pi + qc, 0:16],
                        rhs=sq[:, qc * 512 : (qc + 1) * 512],
                        start=(nmm == 0),
                        stop=(nmm == 7),
                    )
                    nmm += 1
            kvt = scr.tile([16, 512], BF16, tag="kvt")
            lift_times(kvt[:], kvnrm[:], "qk16")
            for pi, dst in enumerate((ksT, vT)):
                for qc in range(4):
                    r = 8 * pi + 2 * qc
                    nc.sync.dma_start(
                        dst[0:65:64, hp, qc * 512 : (qc + 1) * 512],
                        kvt[r : r + 2, :],
                    )
            # V -> natural layout in ONE xbar transpose:
            # transposed row r (= key) lands at v_nat[r%128, r//128, hp, :].
            nc.sync.dma_start(v_nat[:, :, hp, :], vT[:, hp, :], transpose=True)

        def attention_hp(hp):
            pv_tiles[hp] = psPV.tile([P, QB], F32, name=f"pv{hp}", tag=f"pv{hp % 2}")
            for mc in range(16):
                s_ps = psS.tile([P, 1024], F32, tag="s")
                for h in range(2):
                    nc.tensor.matmul(
                        s_ps[:, h * 512 : (h + 1) * 512],
                        lhsT=ksT[h * 64 : (h + 1) * 64, hp, mc * P : (mc + 1) * P],
                        rhs=qsT[h * 64 : (h + 1) * 64, hp, :],
                        start=True,
                        stop=True,
                    )
                p_sb = pP.tile([P, 1024], BF16, tag="p")
                nc.scalar.activation(
                    p_sb[:], s_ps[:], EXP, scale=act_scale, bias=ebias[:]
                )
                for h in range(2):
                    nc.tensor.matmul(
                        pv_tiles[hp][h * 64 : (h + 1) * 64, :],
                        lhsT=v_nat[:, mc, hp, h * 64 : (h + 1) * 64],
                        rhs=p_sb[:, h * 512 : (h + 1) * 512],
                        start=(mc == 0),
                        stop=(mc == 15),
                        skip_group_check=True,
                    )

        o_unT = sb.tile([P, NHP, QB], F32)
        o_nat = sb.tile([P, 4, H, D], F32)  # [q%128, qtile, head, d]
        ph_sq = sb.tile([P, 4, H, D], F32)
        ph_inner = sb.tile([P, 4, H, 1], F32)
        ph_t2 = sb.tile([P, 4, H, 1], F32)

        def tail_hp(hp):
            """Drain PV, transpose to natural layout, DVE part of the per-head
            centroid (no ACT ops -> EXP stream stays uninterrupted)."""
            nc.vector.tensor_copy(out=o_unT[:, hp, :], in_=pv_tiles[hp][:])
            for hh in range(2):
                h = 2 * hp + hh
                for qt in range(4):
                    pt = psU.tile([P, 64], F32, tag="u", name="otp")
                    nc.tensor.transpose(
                        pt[:],
                        o_unT[hh * 64 : (hh + 1) * 64, hp, qt * P : (qt + 1) * P],
                        ident2[hh * 64 : (hh + 1) * 64, :],
                    )
                    nc.vector.tensor_copy(out=o_nat[:, qt, h, :], in_=pt[:])
            hsl = slice(2 * hp, 2 * hp + 2)
            src = o_nat[:, :, hsl, :]
            nc.vector.tensor_tensor(ph_sq[:, :, hsl, :], src, src, MULT)
            nc.vector.tensor_reduce(
                ph_inner[:, :, hsl, 0],
                ph_sq[:, :, hsl, :],
                axis=mybir.AxisListType.X,
                op=ADD,
            )
            nc.vector.tensor_tensor(
                ph_t2[:, :, hsl, :], src[:, :, :, 0:1], src[:, :, :, 0:1], MULT
            )
            nc.vector.tensor_scalar_mul(ph_t2[:, :, hsl, :], ph_t2[:, :, hsl, :], -2.0)
            nc.vector.tensor_tensor(
                ph_inner[:, :, hsl, :], ph_inner[:, :, hsl, :], ph_t2[:, :, hsl, :], ADD
            )

        prologue_hp(0)
        prologue_hp(1)
        attention_hp(0)
        prologue_hp(2)
        attention_hp(1)
        tail_hp(0)
        prologue_hp(3)
        attention_hp(2)
        tail_hp(1)
        attention_hp(3)
        tail_hp(2)
        tail_hp(3)

        # ---- per-head centroid scale, head-sum, final centroid ----
        den = sb.tile([P, 4, H, 1], F32)
        nc.scalar.activation(den[:], ph_inner[:], SQRT, bias=0.0, scale=-1.0)
        rec = sb.tile([P, 4, H, 1], F32)
        nc.vector.reciprocal(rec[:], den[:])
        cent = sb.tile([P, 4, H, D], F32)
        nc.vector.tensor_tensor(
            cent[:], o_nat[:], rec[:].to_broadcast((P, 4, H, D)), MULT
        )
        # head-sum as a contiguous binary tree (strided reduce is slow on DVE)
        c4 = cent[:]  # [P, 4, 8, 64]
        h4 = sb.tile([P, 4, 4, D], F32)
        nc.vector.tensor_tensor(h4[:], c4[:, :, 0:4, :], c4[:, :, 4:8, :], ADD)
        h2 = sb.tile([P, 4, 2, D], F32)
        nc.vector.tensor_tensor(h2[:], h4[:, :, 0:2, :], h4[:, :, 2:4, :], ADD)
        hsum = sb.tile([P, 4, 1, D], F32)
        nc.vector.tensor_tensor(hsum[:], h2[:, :, 0:1, :], h2[:, :, 1:2, :], ADD)
        fsq = sb.tile([P, 4, 1, D], F32)
        nc.vector.tensor_tensor(fsq[:], hsum[:], hsum[:], MULT)
        finner = sb.tile([P, 4, 1, 1], F32)
        nc.vector.tensor_reduce(
            finner[:, :, :, 0], fsq[:], axis=mybir.AxisListType.X, op=ADD
        )
        ft2 = sb.tile([P, 4, 1, 1], F32)
        nc.vector.tensor_tensor(ft2[:], hsum[:, :, :, 0:1], hsum[:, :, :, 0:1], MULT)
        nc.vector.tensor_scalar_mul(ft2[:], ft2[:], -2.0)
        nc.vector.tensor_tensor(finner[:], finner[:], ft2[:], ADD)
        fden = sb.tile([P, 4, 1, 1], F32)
        nc.scalar.activation(fden[:], finner[:], SQRT, bias=0.0, scale=-1.0)
        frec = sb.tile([P, 4, 1, 1], F32)
        nc.vector.reciprocal(frec[:], fden[:])
        out_sb = sb.tile([P, 4, D], F32)
        nc.vector.tensor_tensor(
            out_sb[:],
            hsum[:, :, 0, :],
            frec[:, :, 0, :].to_broadcast((P, 4, D)),
            MULT,
        )
        nc.sync.dma_start(
            io["out"].ap().rearrange("(t p) d -> p t d", p=P), out_sb[:]
        )


def _build(scale_val, bias_val):
    nc = bacc.Bacc(num_devices=N_CORES)
    io = {}
    io["xq_t"] = nc.declare_dram_parameter("xq_t", [P, 4, QB], BF16, isOutput=False)
    for qc in range(4):
        io[f"xs{qc}"] = nc.declare_dram_parameter(
            f"xs{qc}", [P, 4, 512], BF16, isOutput=False
        )
    for nm in ("wq", "wk", "wv"):
        io[nm] = nc.declare_dram_parameter(nm, [P, 4, 4, P], BF16, isOutput=False)
    for nm in ("bq", "bk", "bv"):
        io[nm] = nc.declare_dram_parameter(nm, [P, 4], F32, isOutput=False)
    io["ident2"] = nc.declare_dram_parameter("ident2", [P, 64], F32, isOutput=False)
    io["mask32"] = nc.declare_dram_parameter("mask32", [P, 16, 32], BF16, isOutput=False)
    io["out"] = nc.declare_dram_parameter("out", [QB, D], F32, isOutput=True)

    with tile.TileContext(nc) as tc:
        _emit(tc, nc, io, scale_val, bias_val)
    nc.compile()
    return nc


_BUILD_CACHE = {}


def _get_nc(scale_val, bias_val):
    key = (float(scale_val), float(bias_val))
    if key not in _BUILD_CACHE:
        _BUILD_CACHE[key] = _build(*key)
    return _BUILD_CACHE[key]


def _pad_wT8(w):
    """w: [504, 512] spatial weights for 8 heads -> [512, 512] transposed with
    zero columns at each head's time slot (col h*64)."""
    out = np.zeros((E, 512), dtype=np.float32)
    for h in range(H):
        out[:, h * 64 + 1 : (h + 1) * 64] = w[h * DM1 : (h + 1) * DM1, :].T
    return np.ascontiguousarray(out)


def _pad_b8(b):
    out = np.zeros((512,), dtype=np.float32)
    for h in range(H):
        out[h * 64 + 1 : (h + 1) * 64] = b[h * DM1 : (h + 1) * DM1]
    return out


def _fmt_w(wpad, BF):
    # [E, 512] -> [128 p, 4 ec, 4 pt, 128 m]
    return np.ascontiguousarray(
        wpad.reshape(4, P, 4, P).transpose(1, 0, 2, 3)
    ).astype(BF)


def _fmt_x(x_t, BF):
    # [E, ncols] -> [128 p, 4 ec, ncols]
    return np.ascontiguousarray(
        x_t.reshape(4, P, x_t.shape[1]).transpose(1, 0, 2)
    ).astype(BF)


def make_in_maps(
    query_input, source_input, Wq_w, Wq_b, Wk_w, Wk_b, Wv_w, Wv_b, scale, bias
):
    import ml_dtypes

    BF = ml_dtypes.bfloat16
    ident2 = np.concatenate([np.eye(64), np.eye(64)], axis=0).astype(np.float32)
    mask32 = np.zeros((P, 16, 32), dtype=np.float32)
    for j in range(16):
        mask32[1:64, j, 2 * j] = 1.0
        mask32[65:128, j, 2 * j + 1] = 1.0
    mask32 = mask32.astype(BF)

    wq = _fmt_w(_pad_wT8(Wq_w), BF)
    wk = _fmt_w(_pad_wT8(-Wk_w), BF)  # Lorentz sign folded into K
    wv = _fmt_w(_pad_wT8(Wv_w), BF)
    bq = np.ascontiguousarray(_pad_b8(Wq_b).reshape(4, P).T)
    bk = np.ascontiguousarray(_pad_b8(-Wk_b).reshape(4, P).T)
    bv = np.ascontiguousarray(_pad_b8(Wv_b).reshape(4, P).T)

    xs_chunks = []
    for b in range(B):
        xt = source_input[b].T  # [E, N]
        xs_chunks.append(
            [_fmt_x(xt[:, qc * 512 : (qc + 1) * 512], BF) for qc in range(4)]
        )

    in_maps = []
    for c in range(N_CORES):
        b = c // 4
        g = c % 4
        m = {
            "xq_t": _fmt_x(query_input[b, g * QB : (g + 1) * QB, :].T, BF),
            "wq": wq,
            "wk": wk,
            "wv": wv,
            "bq": bq,
            "bk": bk,
            "bv": bv,
            "ident2": ident2,
            "mask32": mask32,
        }
        for qc in range(4):
            m[f"xs{qc}"] = xs_chunks[b][qc]
        in_maps.append(m)
    return in_maps


def kernel(
    query_input,
    source_input,
    Wq_w,
    Wq_b,
    Wk_w,
    Wk_b,
    Wv_w,
    Wv_b,
    scale,
    bias,
    _trace=False,
):
    scale_val = float(np.asarray(scale).reshape(-1)[0])
    bias_val = float(np.asarray(bias).reshape(-1)[0]) if np.asarray(bias).size else 0.0

    nc = _get_nc(scale_val, bias_val)
    in_maps = make_in_maps(
        query_input, source_input, Wq_w, Wq_b, Wk_w, Wk_b, Wv_w, Wv_b, scale, bias
    )

    from concourse.bass_utils import run_bass_kernel_spmd

    res = run_bass_kernel_spmd(
        nc, in_maps, core_ids=list(range(N_CORES)), trace=_trace
    )

    out = np.zeros((B, N, D), dtype=np.float32)
    for c in range(N_CORES):
        b = c // 4
        g = c % 4
        out[b, g * QB : (g + 1) * QB, :] = res.results[c]["out"]
    if _trace:
        kernel.last_exec_time_ns = res.exec_time_ns
        kernel.last_results = res
    return out
